# revision 7
# baseline (speedup 1.0000x reference)
"""Trainium2 Bass kernel for EnetGnn (gnn_message_passing).

Data-parallel over batch N=8, one sample per NeuronCore. Per-core design:

1. Median pool: host stages negated fp16 blocks in [16, 128, 4, 64] tiles so
   each load is one contiguous 64KB DMA. DVE max8/match_replace rank-32
   rounds; medians collected in SBUF, flattened via one PE transpose + DMA.
2. KNN threshold without indices: e'[i,j] = 2p_i.p_j - |p_j|^2 via K=5 fp16
   matmuls into single-bank psum chunks, ACT-evacuated to fp16 ef. Per-row
   16th-largest te via max8 + mask-removal + max8 (all DVE).
3. S = Sign(ef - te + eps) on the ACT engine with per-row bias (no phase-2
   matmul recompute). S tiles land in one [128, 22, 2720] fp8 SBUF tensor.
4. Aggregation A@gh = (G + S@gh)/2 accumulates per-tile in two long-lived
   3-bank psum halves DURING the p1 loop (PE is otherwise idle there). The
   ones-column gives G per tile; after the loop a rank-1 matmul broadcasts
   G to every column so mts = G + S@gh evacuates at natural scale to fp16.
5. GNN g-MLP/q-update/transposes/conv all fp16 on the PE; q bias is just qb
   (no separate G bias path).
"""
import numpy as np
import concourse.bass as bass
import concourse.bacc as bacc
import concourse.mybir as mybir
import concourse.tile as tile
from concourse.bass_utils import run_bass_kernel_spmd

F32 = mybir.dt.float32
F16 = mybir.dt.float16
F8 = mybir.dt.float8e4
AF = mybir.ActivationFunctionType
ALU = mybir.AluOpType

N, C, H, W = 8, 128, 45, 60
HW = H * W                      # 2700
K = 16
NEG_F16 = -60000.0

# e' evac chunks (cols 0..2700)
CHUNKS6 = [(0, 512), (512, 512), (1024, 512), (1536, 512), (2048, 512), (2560, 140)]
# aggregation chunks incl the ones-column G at col 2700
AGG_CHUNKS = [(0, 512), (512, 512), (1024, 512), (1536, 512), (2048, 512), (2560, 141)]
PTILES = [(t * 128, 128) for t in range(21)] + [(2688, 12)]
# conv row chunks: 5x8 rows + 1x5 rows
RCHUNKS = [(0, 8), (8, 8), (16, 8), (24, 8), (32, 8), (40, 5)]
# transpose groups: 8 + 8 + 6 tiles -> [C, 1024] fp16 psum slots
TGROUPS = [list(range(0, 8)), list(range(8, 16)), list(range(16, 22))]

_cache = {}


def _ensure_ntff_hook():
    import sys
    import types
    try:
        from antenv.axon_hooks import get_axon_ntff_profile_hook  # noqa: F401
        return
    except ImportError:
        pass
    try:
        mod = types.ModuleType("antenv.axon_hooks")
        mod._hook = None

        def set_axon_ntff_profile_hook(h):
            mod._hook = h

        def get_axon_ntff_profile_hook():
            return mod._hook

        mod.set_axon_ntff_profile_hook = set_axon_ntff_profile_hook
        mod.get_axon_ntff_profile_hook = get_axon_ntff_profile_hook
        sys.modules["antenv.axon_hooks"] = mod
        import antenv
        antenv.axon_hooks = mod
        from trn_agent_boot.trn_boot import _ntff_profile_via_ctypes
        hook = _ntff_profile_via_ctypes("/opt/axon/libaxon_pjrt.so")
        if hook is not None:
            mod.set_axon_ntff_profile_hook(hook)
    except Exception as e:  # profiling is best-effort
        print(f"ntff hook injection failed: {e}")


def _build(a0, a1, qa):
    nc = bacc.Bacc("TRN2", target_bir_lowering=False, debug=False, num_devices=8)

    h0_d = nc.dram_tensor("h0", (C, HW), F16, kind="ExternalInput")
    psrcb_d = nc.dram_tensor("psrcb", (16, 128, 4, 64), F16, kind="ExternalInput")
    gw0_d = nc.dram_tensor("gw0T", (C, C), F16, kind="ExternalInput")
    gw1_d = nc.dram_tensor("gw1T", (C, C), F16, kind="ExternalInput")
    qw1_d = nc.dram_tensor("qw1T", (C, C), F16, kind="ExternalInput")
    qw2_d = nc.dram_tensor("qw2T", (C, C), F16, kind="ExternalInput")
    cw_d = nc.dram_tensor("convwT", (C, 18, C), F16, kind="ExternalInput")
    bias_d = nc.dram_tensor("biases", (C, 4), F32, kind="ExternalInput")
    ident_d = nc.dram_tensor("ident", (C, C), F16, kind="ExternalInput")
    uvc_d = nc.dram_tensor("uvc", (2, 8, 2816), F16, kind="ExternalInput")
    out_d = nc.dram_tensor("out", (C, HW), F32, kind="ExternalOutput")

    with tile.TileContext(nc) as tc:
        with tc.tile_pool(name="sb", bufs=1) as sb, \
             tc.tile_pool(name="work", bufs=2) as work, \
             tc.tile_pool(name="ps", bufs=1, space="PSUM") as ps, \
             tc.tile_pool(name="dram", bufs=1, space="DRAM") as dram:

            projn_d = dram.tile([8192], F16, tag="projn_d")

            # median block DMAs first so the DVE phase starts immediately
            blks = []
            for g in range(16):
                blk = work.tile([128, 4, 64], F16, tag="blk", bufs=8,
                                name=f"blk_{g}")
                nc.sync.dma_start(blk[:], psrcb_d[g])
                blks.append(blk)

            # ---------------- persistent SBUF ----------------
            h0 = sb.tile([C, 2720], F16, tag="h0")
            nc.sync.dma_start(h0[:, 0:HW], h0_d[:])
            gw0 = sb.tile([C, C], F16, tag="gw0")
            nc.sync.dma_start(gw0[:], gw0_d[:])
            gw1 = sb.tile([C, C], F16, tag="gw1")
            nc.sync.dma_start(gw1[:], gw1_d[:])
            qw1 = sb.tile([C, C], F16, tag="qw1")
            nc.sync.dma_start(qw1[:], qw1_d[:])
            qw2 = sb.tile([C, C], F16, tag="qw2")
            nc.sync.dma_start(qw2[:], qw2_d[:])
            cw = sb.tile([C, 18, C], F16, tag="cw")
            nc.sync.dma_start(cw[:], cw_d[:])
            bia = sb.tile([C, 4], F32, tag="bias")
            nc.sync.dma_start(bia[:], bias_d[:])
            ident = sb.tile([C, C], F16, tag="ident")
            nc.sync.dma_start(ident[:], ident_d[:])

            U = sb.tile([8, 2816], F16, tag="U")       # [2q; 1; 1]
            nc.sync.dma_start(U[:], uvc_d[0])
            V = sb.tile([8, 2816], F16, tag="V")       # [q; hi; lo]
            nc.sync.dma_start(V[:], uvc_d[1])
            Sbig = sb.tile([C, 22, 2720], F8, tag="Sbig")
            nc.vector.memset(Sbig[:, :, HW:HW + 1], 1.0)   # ones-cols for G
            ghrm = sb.tile([C, 2816], F16, tag="ghrm")
            M8 = sb.tile([C, 64, 8], F16, tag="M8")
            Mt = sb.tile([64, C], F16, tag="Mt")
            onesrow = sb.tile([1, 2816], F16, tag="onesrow")
            nc.vector.memset(onesrow[:], 1.0)

            # ---------------- median pooling (host pre-negated fp16 blocks) --
            for g in range(16):
                blk = blks[g]
                for s in range(4):
                    mm8 = work.tile([128, 8], F16, tag="mm8", bufs=8)
                    for rnd in range(3):
                        nc.vector.max(mm8[:], blk[:, s, :])
                        nc.vector.match_replace(blk[:, s, :], mm8[:], blk[:, s, :], NEG_F16)
                    nc.vector.max(M8[:, g * 4 + s, :], blk[:, s, :])

            # ---------------- iter-1 g-MLP (only needs h0) -------------------
            def mlp_layer(w, h_in, out, it, lab, bias, alpha):
                for half, o0, on in ((0, 0, 1536), (1, 1536, HW - 1536)):
                    gp = ps.tile([C, 1536], F32, tag="aggps", bufs=2,
                                 name=f"{lab}_{it}_{half}")
                    for c0, ncn in (CHUNKS6[:3] if half == 0 else CHUNKS6[3:]):
                        nc.tensor.matmul(gp[:, c0 - o0:c0 - o0 + ncn], w[:],
                                         h_in[:, c0:c0 + ncn], start=True, stop=True)
                    nc.scalar.activation(out[:, o0:o0 + on], gp[:, 0:on], AF.Prelu,
                                         bias=bias, alpha=alpha)

            def gmlp(h_in, it):
                gh1 = work.tile([C, 2720], F16, tag="gh", bufs=2, name=f"gh1_{it}")
                mlp_layer(gw0, h_in, gh1, it, "g1", bia[:, 0:1], a0)
                gh2 = work.tile([C, 2720], F16, tag="gh", bufs=2, name=f"gh2_{it}")
                mlp_layer(gw1, gh1, gh2, it, "g2", bia[:, 1:2], a1)
                return gh2

            def transposes(gh2, it):
                for grp, jts in enumerate(TGROUPS):
                    tp = ps.tile([C, 1024], F16, tag="p1ps", bufs=2,
                                 name=f"tp_{it}_{grp}")
                    for k, jt in enumerate(jts):
                        j0, nj = PTILES[jt]
                        nc.tensor.transpose(tp[0:nj, 128 * k:128 * k + 128],
                                            gh2[:, j0:j0 + nj], ident[:])
                    base = 1024 * grp
                    if grp < 2:
                        nc.scalar.activation(ghrm[:, base:base + 1024],
                                             tp[:, 0:1024], AF.Copy)
                    else:
                        nc.scalar.activation(ghrm[:, base:base + 640],
                                             tp[:, 0:640], AF.Copy)
                        nc.scalar.activation(ghrm[0:12, base + 640:base + 768],
                                             tp[0:12, 640:768], AF.Copy)

            gh2_1 = gmlp(h0, 0)
            transposes(gh2_1, 0)

            # conv pad for h0 half (early)
            pad0 = sb.tile([C, H + 2, W + 2], F16, tag="pad0")
            nc.vector.memset(pad0[:], 0.0)
            nc.scalar.activation(pad0[:, 1:H + 1, 1:W + 1],
                                 h0[:, 0:HW].rearrange("p (h w) -> p h w", h=H), AF.Copy)

            # early h0-half of the conv (9 taps), single-bank psum chunks
            convacc = sb.tile([C, 2720], F32, tag="convacc")
            taps = [(a, b) for a in range(3) for b in range(3)]
            for ri, (r0, nr) in enumerate(RCHUNKS):
                cpe = ps.tile([C, 512], F32, tag="p1ps", bufs=2, name=f"cpe_{ri}")
                for ti, (dy, dx) in enumerate(taps):
                    idx = (dy * 3 + dx) * 2
                    nc.tensor.matmul(cpe[:, 0:nr * W], cw[:, idx, :],
                                     pad0[:, r0 + dy:r0 + dy + nr, dx:dx + W],
                                     start=(ti == 0), stop=(ti == 8))
                nc.scalar.activation(convacc[:, r0 * W:(r0 + nr) * W],
                                     cpe[:, 0:nr * W], AF.Identity, bias=bia[:, 3:4])

            # ---------------- proj flatten via PE transpose ------------------
            mtp = ps.tile([C, 1024], F16, tag="p1ps", bufs=2, name="mtp")
            Mcols = M8[:, :, 7:8].rearrange("p a b -> p (a b)")
            nc.tensor.transpose(mtp[0:64, 0:128], Mcols, ident[:])
            nc.scalar.activation(Mt[:], mtp[0:64, 0:128], AF.Copy)
            projn_r = projn_d.rearrange("(a b) -> a b", b=128)
            nc.sync.dma_start(projn_r[:], Mt[:])

            # U/V staging: q rows (fp16 medians, negated: q = -p)
            for ch in range(3):
                nc.sync.dma_start(V[ch:ch + 1, 0:HW], projn_d[ch * HW:(ch + 1) * HW])
            nc.scalar.activation(U[0:3, 0:HW], V[0:3, 0:HW], AF.Copy, scale=2.0)
            # sq via fp32 Square + ones-matmul
            sq3 = work.tile([3, 2720], F32, tag="bigf32", bufs=1, name="sq3")
            nc.scalar.activation(sq3[:, 0:HW], V[0:3, 0:HW], AF.Square)
            ones3 = sb.tile([3, 1], F32, tag="ones3")
            nc.vector.memset(ones3[:], 1.0)
            sqA = ps.tile([C, 1536], F32, tag="aggps", bufs=2, name="sqA")
            sqB = ps.tile([C, 1536], F32, tag="aggps", bufs=2, name="sqB")
            for c0, ncn in CHUNKS6:
                tgt = sqA[0:1, c0:c0 + ncn] if c0 < 1536 else sqB[0:1, c0 - 1536:c0 - 1536 + ncn]
                nc.tensor.matmul(tgt, ones3[:], sq3[:, c0:c0 + ncn],
                                 start=True, stop=True)
            hirow = work.tile([1, 2816], F16, tag="row", bufs=2, name="hirow")
            lorow = work.tile([1, 2816], F16, tag="row", bufs=2, name="lorow")
            for sq_h, o0, on in ((sqA, 0, 1536), (sqB, 1536, HW - 1536)):
                nc.scalar.activation(hirow[0:1, o0:o0 + on], sq_h[0:1, 0:on],
                                     AF.Copy, scale=-1.0)
                nc.vector.scalar_tensor_tensor(lorow[0:1, o0:o0 + on],
                                               sq_h[0:1, 0:on], -1.0,
                                               hirow[0:1, o0:o0 + on],
                                               ALU.mult, ALU.subtract)
            nc.sync.dma_start(V[3:4, 0:HW], hirow[0:1, 0:HW])
            nc.sync.dma_start(V[4:5, 0:HW], lorow[0:1, 0:HW])

            # ---------------- p1 + sign + aggregation-1, fused per tile ------
            aggA = ps.tile([C, 1536], F32, tag="aggps", bufs=2, name="agg1A")
            aggB = ps.tile([C, 1536], F32, tag="aggps", bufs=2, name="agg1B")

            def agg_tgt(ag, c0, ncn):
                A, B = ag
                if c0 < 1536:
                    return A[:, c0:c0 + ncn]
                return B[:, c0 - 1536:c0 - 1536 + ncn]

            def p1_tile(jt, ag):
                i0, ni = PTILES[jt]
                ef = work.tile([C, 2720], F16, tag="ef", bufs=3, name=f"ef_{jt}")
                for c0, ncn in CHUNKS6:
                    pp = ps.tile([C, 512], F32, tag="p1ps", bufs=2,
                                 name=f"pp_{jt}_{c0}")
                    nc.tensor.matmul(pp[0:ni, 0:ncn], U[0:5, i0:i0 + ni],
                                     V[0:5, c0:c0 + ncn], start=True, stop=True)
                    nc.scalar.activation(ef[0:ni, c0:c0 + ncn], pp[0:ni, 0:ncn],
                                         AF.Copy)
                t8a = work.tile([C, 8], F16, tag="t8", bufs=4, name=f"t8a_{jt}")
                nc.vector.max(t8a[0:ni], ef[0:ni, 0:HW])
                v8f = work.tile([C, 1], F32, tag="v8f", bufs=8, name=f"v8f_{jt}")
                nc.vector.tensor_copy(v8f[0:ni], t8a[0:ni, 7:8])
                msk = work.tile([C, 2720], F16, tag="msk", bufs=2, name=f"msk_{jt}")
                nc.vector.tensor_scalar(msk[0:ni, 0:HW], ef[0:ni, 0:HW],
                                        v8f[0:ni], NEG_F16,
                                        op0=ALU.is_ge, op1=ALU.mult)
                eft = work.tile([C, 2720], F16, tag="msk", bufs=2, name=f"eft_{jt}")
                nc.vector.tensor_tensor(eft[0:ni, 0:HW], ef[0:ni, 0:HW],
                                        msk[0:ni, 0:HW], ALU.add)
                t8b = work.tile([C, 8], F16, tag="t8", bufs=4, name=f"t8b_{jt}")
                nc.vector.max(t8b[0:ni], eft[0:ni, 0:HW])
                # bias = -te + |te|*2^-11 + 4e-7 (eps keeps the te element in)
                tp1 = work.tile([C, 1], F32, tag="v8f", bufs=8, name=f"tp1_{jt}")
                nc.vector.tensor_scalar(tp1[0:ni], t8b[0:ni, 7:8], 2.0 ** -11, 0.0,
                                        op0=ALU.mult, op1=ALU.add)
                tab = work.tile([C, 1], F32, tag="v8f", bufs=8, name=f"tab_{jt}")
                nc.vector.scalar_tensor_tensor(tab[0:ni], t8b[0:ni, 7:8],
                                               -(2.0 ** -11), tp1[0:ni],
                                               ALU.mult, ALU.max)
                bv = work.tile([C, 1], F32, tag="v8f", bufs=8, name=f"bv_{jt}")
                nc.vector.scalar_tensor_tensor(bv[0:ni], tab[0:ni], 4.0e-7,
                                               t8b[0:ni, 7:8], ALU.add, ALU.subtract)
                # S tile via ACT Sign with per-row bias
                nc.scalar.activation(Sbig[0:ni, jt, 0:HW], ef[0:ni, 0:HW],
                                     AF.Sign, bias=bv[0:ni])
                # aggregation contribution of this tile (accumulates in psum)
                for c0, ncn in AGG_CHUNKS:
                    nc.tensor.matmul(agg_tgt(ag, c0, ncn),
                                     ghrm[0:ni, 128 * jt:128 * jt + 128],
                                     Sbig[0:ni, jt, c0:c0 + ncn],
                                     start=(jt == 0), stop=False)

            for jt in range(22):
                p1_tile(jt, (aggA, aggB))

            # G broadcast: extract G column, transpose to a row, rank-1 add
            def g_fix(ag, it):
                A, B = ag
                gcol = sb.tile([C, 1], F16, tag=f"gcol_{it}")
                nc.scalar.activation(gcol[:], B[:, 1164:1165], AF.Copy)
                gpt = ps.tile([C, 1024], F16, tag="p1ps", bufs=2, name=f"gpt_{it}")
                nc.tensor.transpose(gpt[0:1, 0:128], gcol[:, 0:1], ident[:])
                grow = sb.tile([1, C], F16, tag=f"grow_{it}")
                nc.scalar.activation(grow[:], gpt[0:1, 0:128], AF.Copy)
                for ci, (c0, ncn) in enumerate(AGG_CHUNKS):
                    nc.tensor.matmul(agg_tgt(ag, c0, ncn), grow[0:1, 0:C],
                                     onesrow[0:1, c0:c0 + ncn],
                                     start=False, stop=True)

            def mts_evac(ag, it):
                A, B = ag
                mts = work.tile([C, 2720], F16, tag="mts", bufs=1, name=f"mts_{it}")
                nc.scalar.activation(mts[:, 0:1536], A[:, 0:1536], AF.Copy)
                nc.scalar.activation(mts[:, 1536:HW], B[:, 0:1164], AF.Copy)
                return mts

            g_fix((aggA, aggB), 0)
            mts1 = mts_evac((aggA, aggB), 0)

            # ---------------- GNN q-update + iteration 2 ---------------------
            def q_update(h_in, mts, it):
                h_out = work.tile([C, 2720], F16, tag="h", bufs=2, name=f"h_{it}")
                for half, o0, on in ((0, 0, 1536), (1, 1536, HW - 1536)):
                    qp = ps.tile([C, 1536], F32, tag="aggps", bufs=2,
                                 name=f"qp_{it}_{half}")
                    for c0, ncn in (CHUNKS6[:3] if half == 0 else CHUNKS6[3:]):
                        nc.tensor.matmul(qp[:, c0 - o0:c0 - o0 + ncn], qw1[:],
                                         h_in[:, c0:c0 + ncn], start=True, stop=False)
                        nc.tensor.matmul(qp[:, c0 - o0:c0 - o0 + ncn], qw2[:],
                                         mts[:, c0:c0 + ncn], start=False, stop=True)
                    nc.scalar.activation(h_out[:, o0:o0 + on], qp[:, 0:on], AF.Prelu,
                                         bias=bia[:, 2:3], alpha=qa)
                return h_out

            h1 = q_update(h0, mts1, 0)
            gh2_2 = gmlp(h1, 1)
            transposes(gh2_2, 1)

            aggA2 = ps.tile([C, 1536], F32, tag="aggps", bufs=2, name="agg2A")
            aggB2 = ps.tile([C, 1536], F32, tag="aggps", bufs=2, name="agg2B")
            for jt, (j0, nj) in enumerate(PTILES):
                for c0, ncn in AGG_CHUNKS:
                    nc.tensor.matmul(agg_tgt((aggA2, aggB2), c0, ncn),
                                     ghrm[0:nj, 128 * jt:128 * jt + 128],
                                     Sbig[0:nj, jt, c0:c0 + ncn],
                                     start=(jt == 0), stop=False)
            g_fix((aggA2, aggB2), 1)
            mts2 = mts_evac((aggA2, aggB2), 1)
            h2 = q_update(h1, mts2, 1)

            # ---------------- conv 3x3 (h2 half) -----------------------------
            pad1 = sb.tile([C, H + 2, W + 2], F16, tag="pad1")
            nc.vector.memset(pad1[:], 0.0)
            nc.scalar.activation(pad1[:, 1:H + 1, 1:W + 1],
                                 h2[:, 0:HW].rearrange("p (h w) -> p h w", h=H), AF.Copy)
            oc = work.tile([C, 2720], F32, tag="bigf32", bufs=1, name="oc")
            for ri, (r0, nr) in enumerate(RCHUNKS):
                cpe = ps.tile([C, 512], F32, tag="p1ps", bufs=2, name=f"cp2_{ri}")
                for ti, (dy, dx) in enumerate(taps):
                    idx = (dy * 3 + dx) * 2 + 1
                    nc.tensor.matmul(cpe[:, 0:nr * W], cw[:, idx, :],
                                     pad1[:, r0 + dy:r0 + dy + nr, dx:dx + W],
                                     start=(ti == 0), stop=(ti == 8))
                nc.vector.tensor_tensor(oc[:, r0 * W:(r0 + nr) * W],
                                        cpe[:, 0:nr * W],
                                        convacc[:, r0 * W:(r0 + nr) * W], ALU.add)
                if ri == 2:
                    nc.sync.dma_start(out_d[:, 0:1440], oc[:, 0:1440])
                elif ri == 5:
                    nc.sync.dma_start(out_d[:, 1440:2700], oc[:, 1440:2700])

    nc.compile()
    return nc


def kernel(cnn_encoder_output, original_input, xy,
           g_w0, g_b0, g_a0, g_w1, g_b1, g_a1,
           q_w, q_b, q_a, conv_w, conv_b,
           gnn_iterations, k, use_half_precision, _trace=False):
    assert int(gnn_iterations) == 2 and int(k) == 16 and int(use_half_precision) == 0

    cnn = np.asarray(cnn_encoder_output, dtype=np.float32)
    orig = np.asarray(original_input, dtype=np.float32)
    xy = np.asarray(xy, dtype=np.float32)
    a0, a1, qa = float(np.ravel(g_a0)[0]), float(np.ravel(g_a1)[0]), float(np.ravel(q_a)[0])

    key = (a0, a1, qa)
    if key not in _cache:
        _cache[key] = _build(a0, a1, qa)
    nc = _cache[key]

    g_w0 = np.asarray(g_w0, np.float32)
    g_w1 = np.asarray(g_w1, np.float32)
    q_w = np.asarray(q_w, np.float32)
    conv_w = np.asarray(conv_w, np.float32)

    gw0T = np.ascontiguousarray(g_w0.T).astype(np.float16)
    gw1T = np.ascontiguousarray(g_w1.T).astype(np.float16)
    qw1T = np.ascontiguousarray(q_w[:, :C].T).astype(np.float16)
    qw2T = np.ascontiguousarray(q_w[:, C:].T / float(2 * K)).astype(np.float16)
    cwT = np.empty((C, 18, C), np.float16)
    for dy in range(3):
        for dx in range(3):
            for kh in range(2):
                idx = (dy * 3 + dx) * 2 + kh
                cwT[:, idx, :] = conv_w[:, kh * C:(kh + 1) * C, dy, dx].T.astype(np.float16)
    biases = np.stack([np.asarray(g_b0, np.float32), np.asarray(g_b1, np.float32),
                       np.asarray(q_b, np.float32), np.asarray(conv_b, np.float32)],
                      axis=1)
    ident = np.eye(C, dtype=np.float16)
    uvc = np.zeros((2, 8, 2816), np.float16)
    uvc[0, 3:5] = 1.0

    shared = dict(gw0T=gw0T, gw1T=gw1T, qw1T=qw1T, qw2T=qw2T, convwT=cwT,
                  biases=np.ascontiguousarray(biases), ident=ident, uvc=uvc)
    in_maps = []
    for n in range(N):
        # negated fp16 blocks: [3, 2700, 64] -> [16, 128, 4, 64] with
        # block id b = g*512 + s*128 + p  ->  psrcb[g, p, s, :]
        chans = np.stack([xy[n, 0], xy[n, 1], orig[n, 3]], axis=0)      # [3, 360, 480]
        blocks = chans.reshape(3, H, 8, W, 8).transpose(0, 1, 3, 2, 4).reshape(3 * HW, 64)
        blocks = (-blocks).astype(np.float16)
        pad = np.zeros((8192, 64), np.float16)
        pad[:3 * HW] = blocks
        psrcb = pad.reshape(16, 4, 128, 64).transpose(0, 2, 1, 3)
        in_maps.append(dict(h0=np.ascontiguousarray(
                                cnn[n].reshape(C, HW).astype(np.float16)),
                            psrcb=np.ascontiguousarray(psrcb), **shared))

    if _trace:
        _ensure_ntff_hook()
    res = run_bass_kernel_spmd(nc, in_maps, core_ids=list(range(N)), trace=_trace,
                               trace_cores=list(range(N)) if _trace else None)
    out = np.stack([res.results[n]["out"].reshape(C, H, W).astype(np.float32)
                    for n in range(N)])
    if _trace:
        kernel._last_results = res
    return out


# revision 11
# speedup vs baseline: 1.0353x; 1.0353x over previous
"""Trainium2 Bass kernel for EnetGnn (gnn_message_passing).

Data-parallel over batch N=8, one sample per NeuronCore. Per-core design:

1. Median pool: host stages negated fp16 blocks in [16, 128, 4, 64] tiles so
   each load is one contiguous 64KB DMA. DVE max8/match_replace rank-32
   rounds; medians flattened via two half PE transposes + DMA so the x
   channel stages while the second half of the median still runs.
2. KNN threshold: e'[i,j] = 2p_i.p_j - |p_j|^2 via K=4 fp16 matmuls into
   double-buffered 3-bank psum halves, ACT-evacuated to fp16 ef. Per-row
   16th-largest te via max8 + is_ge mask removal + max8 (all DVE).
3. S = Sign(ef - te + eps) on the ACT engine with per-row bias (no phase-2
   matmul recompute). S tiles land in one [128, 22, 2720] fp8 SBUF tensor
   with a ones-column for G.
4. All heavy matmul work (aggregation iters 1+2, q updates, g-MLP 2, conv)
   runs as one dense tail stream to keep the PE at its hot clock. The
   ones-column gives G; a rank-1 matmul broadcasts G so mts = G + S@gh
   evacuates at natural scale to fp16, making the q update all-fp16.
"""
import numpy as np
import concourse.bass as bass
import concourse.bacc as bacc
import concourse.mybir as mybir
import concourse.tile as tile
from concourse.bass_utils import run_bass_kernel_spmd

F32 = mybir.dt.float32
F16 = mybir.dt.float16
F8 = mybir.dt.float8e4
AF = mybir.ActivationFunctionType
ALU = mybir.AluOpType

N, C, H, W = 8, 128, 45, 60
HW = H * W                      # 2700
K = 16
NEG_F16 = -60000.0

CHUNKS6 = [(0, 512), (512, 512), (1024, 512), (1536, 512), (2048, 512), (2560, 140)]
AGG_CHUNKS = [(0, 512), (512, 512), (1024, 512), (1536, 512), (2048, 512), (2560, 141)]
PTILES = [(t * 128, 128) for t in range(21)] + [(2688, 12)]
RCHUNKS = [(0, 8), (8, 8), (16, 8), (24, 8), (32, 8), (40, 5)]
TGROUPS = [list(range(0, 8)), list(range(8, 16)), list(range(16, 22))]

_cache = {}


def _ensure_ntff_hook():
    import sys
    import types
    try:
        from antenv.axon_hooks import get_axon_ntff_profile_hook  # noqa: F401
        return
    except ImportError:
        pass
    try:
        mod = types.ModuleType("antenv.axon_hooks")
        mod._hook = None

        def set_axon_ntff_profile_hook(h):
            mod._hook = h

        def get_axon_ntff_profile_hook():
            return mod._hook

        mod.set_axon_ntff_profile_hook = set_axon_ntff_profile_hook
        mod.get_axon_ntff_profile_hook = get_axon_ntff_profile_hook
        sys.modules["antenv.axon_hooks"] = mod
        import antenv
        antenv.axon_hooks = mod
        from trn_agent_boot.trn_boot import _ntff_profile_via_ctypes
        hook = _ntff_profile_via_ctypes("/opt/axon/libaxon_pjrt.so")
        if hook is not None:
            mod.set_axon_ntff_profile_hook(hook)
    except Exception as e:  # profiling is best-effort
        print(f"ntff hook injection failed: {e}")


def _build(a0, a1, qa):
    nc = bacc.Bacc("TRN2", target_bir_lowering=False, debug=False, num_devices=8)

    h0_d = nc.dram_tensor("h0", (C, HW), F16, kind="ExternalInput")
    psrcb_d = nc.dram_tensor("psrcb", (16, 128, 4, 64), F16, kind="ExternalInput")
    gw0_d = nc.dram_tensor("gw0T", (C, C), F16, kind="ExternalInput")
    gw1_d = nc.dram_tensor("gw1T", (C, C), F16, kind="ExternalInput")
    qw1_d = nc.dram_tensor("qw1T", (C, C), F16, kind="ExternalInput")
    qw2_d = nc.dram_tensor("qw2T", (C, C), F16, kind="ExternalInput")
    cw_d = nc.dram_tensor("convwT", (C, 18, C), F16, kind="ExternalInput")
    bias_d = nc.dram_tensor("biases", (C, 4), F32, kind="ExternalInput")
    ident_d = nc.dram_tensor("ident", (C, C), F16, kind="ExternalInput")
    uvc_d = nc.dram_tensor("uvc", (2, 8, 2816), F16, kind="ExternalInput")
    out_d = nc.dram_tensor("out", (C, HW), F32, kind="ExternalOutput")

    with tile.TileContext(nc) as tc:
        with tc.tile_pool(name="sb", bufs=1) as sb, \
             tc.tile_pool(name="work", bufs=2) as work, \
             tc.tile_pool(name="ps", bufs=1, space="PSUM") as ps, \
             tc.tile_pool(name="dram", bufs=1, space="DRAM") as dram:

            projn_d = dram.tile([8192], F16, tag="projn_d")

            # median block DMAs first so the DVE phase starts immediately
            blks = []
            for g in range(16):
                blk = work.tile([128, 4, 64], F16, tag="blk", bufs=8,
                                name=f"blk_{g}")
                nc.sync.dma_start(blk[:], psrcb_d[g])
                blks.append(blk)

            # ---------------- persistent SBUF ----------------
            h0 = sb.tile([C, 2720], F16, tag="h0")
            nc.sync.dma_start(h0[:, 0:HW], h0_d[:])
            gw0 = sb.tile([C, C], F16, tag="gw0")
            nc.sync.dma_start(gw0[:], gw0_d[:])
            gw1 = sb.tile([C, C], F16, tag="gw1")
            nc.sync.dma_start(gw1[:], gw1_d[:])
            qw1 = sb.tile([C, C], F16, tag="qw1")
            nc.sync.dma_start(qw1[:], qw1_d[:])
            qw2 = sb.tile([C, C], F16, tag="qw2")
            nc.sync.dma_start(qw2[:], qw2_d[:])
            cw = sb.tile([C, 18, C], F16, tag="cw")
            nc.sync.dma_start(cw[:], cw_d[:])
            bia = sb.tile([C, 4], F32, tag="bias")
            nc.sync.dma_start(bia[:], bias_d[:])
            ident = sb.tile([C, C], F16, tag="ident")
            nc.sync.dma_start(ident[:], ident_d[:])

            U = sb.tile([8, 2816], F16, tag="U")       # [2q; 1]
            nc.sync.dma_start(U[:], uvc_d[0])
            V = sb.tile([8, 2816], F16, tag="V")       # [q; -|p|^2]
            nc.sync.dma_start(V[:], uvc_d[1])
            Sbig = sb.tile([C, 22, 2720], F8, tag="Sbig")
            ghrm = sb.tile([C, 2816], F16, tag="ghrm")
            M8 = sb.tile([C, 64, 8], F16, tag="M8")
            Mt = sb.tile([64, C], F16, tag="Mt")
            onesrow = sb.tile([1, 2816], F16, tag="onesrow")
            pad0 = sb.tile([C, H + 2, W + 2], F16, tag="pad0")
            pad1 = sb.tile([C, H + 2, W + 2], F16, tag="pad1")
            convacc = sb.tile([C, 2720], F32, tag="convacc")
            ones3 = sb.tile([3, 1], F16, tag="ones3")

            # memsets on gpsimd (DVE stays on the median path)
            nc.gpsimd.memset(Sbig[:, :, HW:HW + 1], 1.0)   # ones-cols for G
            nc.gpsimd.memset(onesrow[:], 1.0)
            nc.gpsimd.memset(pad0[:], 0.0)
            nc.gpsimd.memset(pad1[:], 0.0)
            nc.gpsimd.memset(ones3[:], 1.0)

            # ---------------- median pooling + split flatten -----------------
            def median_range(glo, ghi):
                for g in range(glo, ghi):
                    blk = blks[g]
                    for s in range(4):
                        mm8 = work.tile([128, 8], F16, tag="mm8", bufs=8)
                        for rnd in range(3):
                            nc.vector.max(mm8[:], blk[:, s, :])
                            nc.vector.match_replace(blk[:, s, :], mm8[:],
                                                    blk[:, s, :], NEG_F16)
                        nc.vector.max(M8[:, g * 4 + s, :], blk[:, s, :])

            def flatten_half(half):
                lo, nc_ = (0, 32) if half == 0 else (32, 32)
                mtp = ps.tile([C, 1024], F16, tag="sm", bufs=2, name=f"mtp{half}")
                Mcols = M8[:, lo:lo + 32, 7:8].rearrange("p a b -> p (a b)")
                nc.tensor.transpose(mtp[0:32, 0:128], Mcols, ident[:])
                nc.scalar.activation(Mt[lo:lo + 32, :], mtp[0:32, 0:128], AF.Copy)
                projn_r = projn_d.rearrange("(a b) -> a b", b=128)
                nc.sync.dma_start(projn_r[lo:lo + 32, :], Mt[lo:lo + 32, :])

            median_range(0, 8)
            flatten_half(0)
            # x channel DMA overlaps the second median half
            nc.sync.dma_start(V[0:1, 0:HW], projn_d[0:HW])
            sq3 = work.tile([3, 2720], F16, tag="sq3", bufs=1, name="sq3")

            median_range(8, 16)
            flatten_half(1)
            for ch in (1, 2):
                nc.sync.dma_start(V[ch:ch + 1, 0:HW], projn_d[ch * HW:(ch + 1) * HW])
            nc.scalar.activation(U[0:3, 0:HW], V[0:3, 0:HW], AF.Copy, scale=2.0)
            nc.scalar.activation(sq3[0:3, 0:HW], V[0:3, 0:HW], AF.Square)
            sqp = ps.tile([C, 1536], F32, tag="big3", bufs=2, name="sqp")
            for c0, ncn in CHUNKS6[:3]:
                nc.tensor.matmul(sqp[0:1, c0:c0 + ncn], ones3[:],
                                 sq3[:, c0:c0 + ncn], start=True, stop=True)
            hirow = work.tile([1, 2816], F16, tag="row", bufs=2, name="hirow")
            nc.scalar.activation(hirow[0:1, 0:1536], sqp[0:1, 0:1536],
                                 AF.Copy, scale=-1.0)
            sqp2 = ps.tile([C, 1536], F32, tag="big3", bufs=2, name="sqp2")
            for c0, ncn in CHUNKS6[3:]:
                nc.tensor.matmul(sqp2[0:1, c0 - 1536:c0 - 1536 + ncn],
                                 ones3[:], sq3[:, c0:c0 + ncn], start=True, stop=True)
            nc.scalar.activation(hirow[0:1, 1536:HW], sqp2[0:1, 0:HW - 1536],
                                 AF.Copy, scale=-1.0)
            nc.sync.dma_start(V[3:4, 0:HW], hirow[0:1, 0:HW])

            # ---------------- iter-1 g-MLP + conv h0-half (under median) -----
            def mlp_layer(w, h_in, out, it, lab, bias, alpha):
                for half, o0, on in ((0, 0, 1536), (1, 1536, HW - 1536)):
                    gp = ps.tile([C, 1536], F32, tag="big3", bufs=2,
                                 name=f"{lab}_{it}_{half}")
                    for c0, ncn in (CHUNKS6[:3] if half == 0 else CHUNKS6[3:]):
                        nc.tensor.matmul(gp[:, c0 - o0:c0 - o0 + ncn], w[:],
                                         h_in[:, c0:c0 + ncn], start=True, stop=True)
                    nc.scalar.activation(out[:, o0:o0 + on], gp[:, 0:on], AF.Prelu,
                                         bias=bias, alpha=alpha)

            def gmlp(h_in, it):
                gh1 = work.tile([C, 2720], F16, tag="gh", bufs=2, name=f"gh1_{it}")
                mlp_layer(gw0, h_in, gh1, it, "g1", bia[:, 0:1], a0)
                gh2 = work.tile([C, 2720], F16, tag="gh", bufs=2, name=f"gh2_{it}")
                mlp_layer(gw1, gh1, gh2, it, "g2", bia[:, 1:2], a1)
                return gh2

            def transposes(gh2, it):
                for grp, jts in enumerate(TGROUPS):
                    tp = ps.tile([C, 1024], F16, tag="sm", bufs=2,
                                 name=f"tp_{it}_{grp}")
                    for k, jt in enumerate(jts):
                        j0, nj = PTILES[jt]
                        nc.tensor.transpose(tp[0:nj, 128 * k:128 * k + 128],
                                            gh2[:, j0:j0 + nj], ident[:])
                    base = 1024 * grp
                    if grp < 2:
                        nc.scalar.activation(ghrm[:, base:base + 1024],
                                             tp[:, 0:1024], AF.Copy)
                    else:
                        nc.scalar.activation(ghrm[:, base:base + 640],
                                             tp[:, 0:640], AF.Copy)
                        nc.scalar.activation(ghrm[0:12, base + 640:base + 768],
                                             tp[0:12, 640:768], AF.Copy)

            gh2_1 = gmlp(h0, 0)
            transposes(gh2_1, 0)

            nc.scalar.activation(pad0[:, 1:H + 1, 1:W + 1],
                                 h0[:, 0:HW].rearrange("p (h w) -> p h w", h=H), AF.Copy)
            taps = [(a, b) for a in range(3) for b in range(3)]
            for ri, (r0, nr) in enumerate(RCHUNKS):
                cpe = ps.tile([C, 512], F32, tag="sm", bufs=2, name=f"cpe_{ri}")
                for ti, (dy, dx) in enumerate(taps):
                    idx = (dy * 3 + dx) * 2
                    nc.tensor.matmul(cpe[:, 0:nr * W], cw[:, idx, :],
                                     pad0[:, r0 + dy:r0 + dy + nr, dx:dx + W],
                                     start=(ti == 0), stop=(ti == 8))
                nc.scalar.activation(convacc[:, r0 * W:(r0 + nr) * W],
                                     cpe[:, 0:nr * W], AF.Identity, bias=bia[:, 3:4])

            # ---------------- p1: per-row te + sign ------------------------
            def p1_tile(jt):
                i0, ni = PTILES[jt]
                ef = work.tile([C, 2720], F16, tag="ef", bufs=3, name=f"ef_{jt}")
                for half, o0, on in ((0, 0, 1536), (1, 1536, HW - 1536)):
                    pp = ps.tile([C, 1536], F32, tag="big3", bufs=2,
                                 name=f"pp_{jt}_{half}")
                    for c0, ncn in (CHUNKS6[:3] if half == 0 else CHUNKS6[3:]):
                        nc.tensor.matmul(pp[0:ni, c0 - o0:c0 - o0 + ncn],
                                         U[0:4, i0:i0 + ni], V[0:4, c0:c0 + ncn],
                                         start=True, stop=True)
                    nc.scalar.activation(ef[0:ni, o0:o0 + on], pp[0:ni, 0:on],
                                         AF.Copy)
                t8a = work.tile([C, 8], F16, tag="t8", bufs=4, name=f"t8a_{jt}")
                nc.vector.max(t8a[0:ni], ef[0:ni, 0:HW])
                v8f = work.tile([C, 1], F32, tag="v8f", bufs=8, name=f"v8f_{jt}")
                nc.vector.tensor_copy(v8f[0:ni], t8a[0:ni, 7:8])
                msk = work.tile([C, 2720], F16, tag="msk", bufs=2, name=f"msk_{jt}")
                nc.vector.tensor_scalar(msk[0:ni, 0:HW], ef[0:ni, 0:HW],
                                        v8f[0:ni], NEG_F16,
                                        op0=ALU.is_ge, op1=ALU.mult)
                eft = work.tile([C, 2720], F16, tag="msk", bufs=2, name=f"eft_{jt}")
                nc.vector.tensor_tensor(eft[0:ni, 0:HW], ef[0:ni, 0:HW],
                                        msk[0:ni, 0:HW], ALU.add)
                t8b = work.tile([C, 8], F16, tag="t8", bufs=4, name=f"t8b_{jt}")
                nc.vector.max(t8b[0:ni], eft[0:ni, 0:HW])
                # bias = -te + |te|*2^-11 + 4e-7
                tp1 = work.tile([C, 1], F32, tag="v8f", bufs=8, name=f"tp1_{jt}")
                nc.vector.tensor_scalar(tp1[0:ni], t8b[0:ni, 7:8], 2.0 ** -11, 0.0,
                                        op0=ALU.mult, op1=ALU.add)
                tab = work.tile([C, 1], F32, tag="v8f", bufs=8, name=f"tab_{jt}")
                nc.vector.scalar_tensor_tensor(tab[0:ni], t8b[0:ni, 7:8],
                                               -(2.0 ** -11), tp1[0:ni],
                                               ALU.mult, ALU.max)
                bv = work.tile([C, 1], F32, tag="v8f", bufs=8, name=f"bv_{jt}")
                nc.vector.scalar_tensor_tensor(bv[0:ni], tab[0:ni], 4.0e-7,
                                               t8b[0:ni, 7:8], ALU.add, ALU.subtract)
                nc.scalar.activation(Sbig[0:ni, jt, 0:HW], ef[0:ni, 0:HW],
                                     AF.Sign, bias=bv[0:ni])

            for jt in range(22):
                p1_tile(jt)

            # ---------------- dense tail: agg1, q1, gmlp2, agg2, q2, conv ----
            def agg_block(it):
                A = ps.tile([C, 1536], F32, tag="big3", bufs=2, name=f"agg{it}A")
                B = ps.tile([C, 1536], F32, tag="big3", bufs=2, name=f"agg{it}B")

                def tgt(c0, ncn):
                    return A[:, c0:c0 + ncn] if c0 < 1536 else B[:, c0 - 1536:c0 - 1536 + ncn]

                for jt, (j0, nj) in enumerate(PTILES):
                    for c0, ncn in AGG_CHUNKS:
                        nc.tensor.matmul(tgt(c0, ncn),
                                         ghrm[0:nj, 128 * jt:128 * jt + 128],
                                         Sbig[0:nj, jt, c0:c0 + ncn],
                                         start=(jt == 0), stop=False)
                # G broadcast: extract G col, transpose to a row, rank-1 add
                gcol = sb.tile([C, 1], F16, tag=f"gcol_{it}")
                nc.scalar.activation(gcol[:], B[:, 1164:1165], AF.Copy)
                gpt = ps.tile([C, 1024], F16, tag="sm", bufs=2, name=f"gpt_{it}")
                nc.tensor.transpose(gpt[0:1, 0:128], gcol[:, 0:1], ident[:])
                grow = sb.tile([1, C], F16, tag=f"grow_{it}")
                nc.scalar.activation(grow[:], gpt[0:1, 0:128], AF.Copy)
                for c0, ncn in AGG_CHUNKS:
                    nc.tensor.matmul(tgt(c0, ncn), grow[0:1, 0:C],
                                     onesrow[0:1, c0:c0 + ncn],
                                     start=False, stop=True)
                mts = work.tile([C, 2720], F16, tag="mts", bufs=1, name=f"mts_{it}")
                nc.scalar.activation(mts[:, 0:1536], A[:, 0:1536], AF.Copy)
                nc.scalar.activation(mts[:, 1536:HW], B[:, 0:1164], AF.Copy)
                return mts

            def q_update(h_in, mts, it):
                h_out = work.tile([C, 2720], F16, tag="h", bufs=2, name=f"h_{it}")
                for half, o0, on in ((0, 0, 1536), (1, 1536, HW - 1536)):
                    qp = ps.tile([C, 1536], F32, tag="big3", bufs=2,
                                 name=f"qp_{it}_{half}")
                    for c0, ncn in (CHUNKS6[:3] if half == 0 else CHUNKS6[3:]):
                        nc.tensor.matmul(qp[:, c0 - o0:c0 - o0 + ncn], qw1[:],
                                         h_in[:, c0:c0 + ncn], start=True, stop=False)
                        nc.tensor.matmul(qp[:, c0 - o0:c0 - o0 + ncn], qw2[:],
                                         mts[:, c0:c0 + ncn], start=False, stop=True)
                    nc.scalar.activation(h_out[:, o0:o0 + on], qp[:, 0:on], AF.Prelu,
                                         bias=bia[:, 2:3], alpha=qa)
                return h_out

            mts1 = agg_block(0)
            h1 = q_update(h0, mts1, 0)
            gh2_2 = gmlp(h1, 1)
            transposes(gh2_2, 1)
            mts2 = agg_block(1)
            h2 = q_update(h1, mts2, 1)

            # conv h2-half
            nc.scalar.activation(pad1[:, 1:H + 1, 1:W + 1],
                                 h2[:, 0:HW].rearrange("p (h w) -> p h w", h=H), AF.Copy)
            oc = work.tile([C, 2720], F32, tag="bigf32", bufs=1, name="oc")
            for ri, (r0, nr) in enumerate(RCHUNKS):
                cpe = ps.tile([C, 512], F32, tag="sm", bufs=2, name=f"cp2_{ri}")
                for ti, (dy, dx) in enumerate(taps):
                    idx = (dy * 3 + dx) * 2 + 1
                    nc.tensor.matmul(cpe[:, 0:nr * W], cw[:, idx, :],
                                     pad1[:, r0 + dy:r0 + dy + nr, dx:dx + W],
                                     start=(ti == 0), stop=(ti == 8))
                nc.vector.tensor_tensor(oc[:, r0 * W:(r0 + nr) * W],
                                        cpe[:, 0:nr * W],
                                        convacc[:, r0 * W:(r0 + nr) * W], ALU.add)
                if ri == 2:
                    nc.sync.dma_start(out_d[:, 0:1440], oc[:, 0:1440])
                elif ri == 5:
                    nc.sync.dma_start(out_d[:, 1440:2700], oc[:, 1440:2700])

    nc.compile()
    return nc


def kernel(cnn_encoder_output, original_input, xy,
           g_w0, g_b0, g_a0, g_w1, g_b1, g_a1,
           q_w, q_b, q_a, conv_w, conv_b,
           gnn_iterations, k, use_half_precision, _trace=False):
    assert int(gnn_iterations) == 2 and int(k) == 16 and int(use_half_precision) == 0

    cnn = np.asarray(cnn_encoder_output, dtype=np.float32)
    orig = np.asarray(original_input, dtype=np.float32)
    xy = np.asarray(xy, dtype=np.float32)
    a0, a1, qa = float(np.ravel(g_a0)[0]), float(np.ravel(g_a1)[0]), float(np.ravel(q_a)[0])

    key = (a0, a1, qa)
    if key not in _cache:
        _cache[key] = _build(a0, a1, qa)
    nc = _cache[key]

    g_w0 = np.asarray(g_w0, np.float32)
    g_w1 = np.asarray(g_w1, np.float32)
    q_w = np.asarray(q_w, np.float32)
    conv_w = np.asarray(conv_w, np.float32)

    gw0T = np.ascontiguousarray(g_w0.T).astype(np.float16)
    gw1T = np.ascontiguousarray(g_w1.T).astype(np.float16)
    qw1T = np.ascontiguousarray(q_w[:, :C].T).astype(np.float16)
    qw2T = np.ascontiguousarray(q_w[:, C:].T / float(2 * K)).astype(np.float16)
    cwT = np.empty((C, 18, C), np.float16)
    for dy in range(3):
        for dx in range(3):
            for kh in range(2):
                idx = (dy * 3 + dx) * 2 + kh
                cwT[:, idx, :] = conv_w[:, kh * C:(kh + 1) * C, dy, dx].T.astype(np.float16)
    biases = np.stack([np.asarray(g_b0, np.float32), np.asarray(g_b1, np.float32),
                       np.asarray(q_b, np.float32), np.asarray(conv_b, np.float32)],
                      axis=1)
    ident = np.eye(C, dtype=np.float16)
    uvc = np.zeros((2, 8, 2816), np.float16)
    uvc[0, 3] = 1.0

    shared = dict(gw0T=gw0T, gw1T=gw1T, qw1T=qw1T, qw2T=qw2T, convwT=cwT,
                  biases=np.ascontiguousarray(biases), ident=ident, uvc=uvc)
    in_maps = []
    for n in range(N):
        chans = np.stack([xy[n, 0], xy[n, 1], orig[n, 3]], axis=0)      # [3, 360, 480]
        blocks = chans.reshape(3, H, 8, W, 8).transpose(0, 1, 3, 2, 4).reshape(3 * HW, 64)
        blocks = (-blocks).astype(np.float16)
        pad = np.zeros((8192, 64), np.float16)
        pad[:3 * HW] = blocks
        psrcb = pad.reshape(16, 4, 128, 64).transpose(0, 2, 1, 3)
        in_maps.append(dict(h0=np.ascontiguousarray(
                                cnn[n].reshape(C, HW).astype(np.float16)),
                            psrcb=np.ascontiguousarray(psrcb), **shared))

    if _trace:
        _ensure_ntff_hook()
    res = run_bass_kernel_spmd(nc, in_maps, core_ids=list(range(N)), trace=_trace,
                               trace_cores=list(range(N)) if _trace else None)
    out = np.stack([res.results[n]["out"].reshape(C, H, W).astype(np.float32)
                    for n in range(N)])
    if _trace:
        kernel._last_results = res
    return out


# revision 14
# speedup vs baseline: 1.2798x; 1.2362x over previous
"""Trainium2 Bass kernel for EnetGnn (gnn_message_passing).

Data-parallel over batch N=8, one sample per NeuronCore. Per-core design:

1. Median pool: host stages negated fp16 blocks in [16, 128, 4, 64] tiles so
   each load is one contiguous 64KB DMA. DVE max8/match_replace rank-32
   rounds; medians flattened via two half PE transposes + DMA so the x
   channel stages while the second half of the median still runs.
2. KNN threshold: e'[i,j] = 2p_i.p_j - |p_j|^2 via K=4 fp16 matmuls into
   double-buffered 3-bank psum halves, ACT-evacuated to fp16 ef. Per-row
   16th-largest te via max8 + is_ge mask removal + max8 (all DVE).
3. S = Sign(ef - te + eps) on the ACT engine with per-row bias (no phase-2
   matmul recompute). S tiles land in one [128, 22, 2720] fp8 SBUF tensor
   with a ones-column for G.
4. All heavy matmul work (aggregation iters 1+2, q updates, g-MLP 2, conv)
   runs as one dense tail stream to keep the PE at its hot clock. The
   ones-column gives G; a rank-1 matmul broadcasts G so mts = G + S@gh
   evacuates at natural scale to fp16, making the q update all-fp16.
"""
import numpy as np
import concourse.bass as bass
import concourse.bacc as bacc
import concourse.mybir as mybir
import concourse.tile as tile
from concourse.bass_utils import run_bass_kernel_spmd

F32 = mybir.dt.float32
F16 = mybir.dt.float16
F8 = mybir.dt.float8e4
AF = mybir.ActivationFunctionType
ALU = mybir.AluOpType

N, C, H, W = 8, 128, 45, 60
HW = H * W                      # 2700
K = 16
NEG_F16 = -60000.0

CHUNKS6 = [(0, 512), (512, 512), (1024, 512), (1536, 512), (2048, 512), (2560, 140)]
AGG_CHUNKS = [(0, 512), (512, 512), (1024, 512), (1536, 512), (2048, 512), (2560, 141)]
PTILES = [(t * 128, 128) for t in range(21)] + [(2688, 12)]
RCHUNKS = [(0, 8), (8, 8), (16, 8), (24, 8), (32, 8), (40, 5)]
TGROUPS = [list(range(0, 8)), list(range(8, 16)), list(range(16, 22))]

_cache = {}


def _ensure_ntff_hook():
    import sys
    import types
    try:
        from antenv.axon_hooks import get_axon_ntff_profile_hook  # noqa: F401
        return
    except ImportError:
        pass
    try:
        mod = types.ModuleType("antenv.axon_hooks")
        mod._hook = None

        def set_axon_ntff_profile_hook(h):
            mod._hook = h

        def get_axon_ntff_profile_hook():
            return mod._hook

        mod.set_axon_ntff_profile_hook = set_axon_ntff_profile_hook
        mod.get_axon_ntff_profile_hook = get_axon_ntff_profile_hook
        sys.modules["antenv.axon_hooks"] = mod
        import antenv
        antenv.axon_hooks = mod
        from trn_agent_boot.trn_boot import _ntff_profile_via_ctypes
        hook = _ntff_profile_via_ctypes("/opt/axon/libaxon_pjrt.so")
        if hook is not None:
            mod.set_axon_ntff_profile_hook(hook)
    except Exception as e:  # profiling is best-effort
        print(f"ntff hook injection failed: {e}")


def _build(a0, a1, qa):
    nc = bacc.Bacc("TRN2", target_bir_lowering=False, debug=False, num_devices=8)

    h0_d = nc.dram_tensor("h0", (C, HW), F16, kind="ExternalInput")
    psrcb_d = nc.dram_tensor("psrcb", (16, 128, 4, 64), F16, kind="ExternalInput")
    gw0_d = nc.dram_tensor("gw0T", (C, C), F16, kind="ExternalInput")
    gw1_d = nc.dram_tensor("gw1T", (C, C), F16, kind="ExternalInput")
    qw1_d = nc.dram_tensor("qw1T", (C, C), F16, kind="ExternalInput")
    qw2_d = nc.dram_tensor("qw2T", (C, C), F16, kind="ExternalInput")
    cw_d = nc.dram_tensor("convwT", (C, 18, C), F16, kind="ExternalInput")
    bias_d = nc.dram_tensor("biases", (C, 4), F32, kind="ExternalInput")
    ident_d = nc.dram_tensor("ident", (C, C), F16, kind="ExternalInput")
    uvc_d = nc.dram_tensor("uvc", (2, 8, 2816), F16, kind="ExternalInput")
    out_d = nc.dram_tensor("out", (C, HW), F32, kind="ExternalOutput")

    with tile.TileContext(nc) as tc:
        with tc.tile_pool(name="sb", bufs=1) as sb, \
             tc.tile_pool(name="work", bufs=2) as work, \
             tc.tile_pool(name="ps", bufs=1, space="PSUM") as ps, \
             tc.tile_pool(name="dram", bufs=1, space="DRAM") as dram:

            projn_d = dram.tile([8192], F16, tag="projn_d")

            # median block DMAs first so the DVE phase starts immediately
            blks = []
            for g in range(16):
                blk = work.tile([128, 4, 64], F16, tag="blk", bufs=8,
                                name=f"blk_{g}")
                nc.sync.dma_start(blk[:], psrcb_d[g])
                blks.append(blk)

            # ---------------- persistent SBUF ----------------
            h0 = sb.tile([C, 2720], F16, tag="h0")
            nc.sync.dma_start(h0[:, 0:HW], h0_d[:])
            gw0 = sb.tile([C, C], F16, tag="gw0")
            nc.sync.dma_start(gw0[:], gw0_d[:])
            gw1 = sb.tile([C, C], F16, tag="gw1")
            nc.sync.dma_start(gw1[:], gw1_d[:])
            qw1 = sb.tile([C, C], F16, tag="qw1")
            nc.sync.dma_start(qw1[:], qw1_d[:])
            qw2 = sb.tile([C, C], F16, tag="qw2")
            nc.sync.dma_start(qw2[:], qw2_d[:])
            cw = sb.tile([C, 18, C], F16, tag="cw")
            nc.sync.dma_start(cw[:], cw_d[:])
            bia = sb.tile([C, 4], F32, tag="bias")
            nc.sync.dma_start(bia[:], bias_d[:])
            ident = sb.tile([C, C], F16, tag="ident")
            nc.sync.dma_start(ident[:], ident_d[:])

            U = sb.tile([8, 2816], F16, tag="U")       # [2q; 1]
            nc.sync.dma_start(U[:], uvc_d[0])
            V = sb.tile([8, 2816], F16, tag="V")       # [q; -|p|^2]
            nc.sync.dma_start(V[:], uvc_d[1])
            Sbig = sb.tile([C, 22, 2720], F8, tag="Sbig")
            ghrm = sb.tile([C, 2816], F16, tag="ghrm")
            M8 = sb.tile([C, 64, 8], F16, tag="M8")
            Mt = sb.tile([64, C], F16, tag="Mt")
            onesrow = sb.tile([1, 2816], F16, tag="onesrow")
            pad0 = sb.tile([C, H + 2, W + 2], F16, tag="pad0")
            pad1 = sb.tile([C, H + 2, W + 2], F16, tag="pad1")
            convacc = sb.tile([C, 2720], F32, tag="convacc")
            ones3 = sb.tile([3, 1], F16, tag="ones3")

            # memsets on gpsimd (DVE stays on the median path)
            nc.gpsimd.memset(Sbig[:, :, HW:HW + 1], 1.0)   # ones-cols for G
            nc.gpsimd.memset(onesrow[:], 1.0)
            nc.gpsimd.memset(pad0[:], 0.0)
            nc.gpsimd.memset(pad1[:], 0.0)
            nc.gpsimd.memset(ones3[:], 1.0)

            # ---------------- median pooling + split flatten -----------------
            def median_range(glo, ghi):
                for g in range(glo, ghi):
                    blk = blks[g]
                    for s in range(4):
                        mm8 = work.tile([128, 8], F16, tag="mm8", bufs=8)
                        for rnd in range(3):
                            nc.vector.max(mm8[:], blk[:, s, :])
                            nc.vector.match_replace(blk[:, s, :], mm8[:],
                                                    blk[:, s, :], NEG_F16)
                        nc.vector.max(M8[:, g * 4 + s, :], blk[:, s, :])

            def flatten_half(half):
                lo, nc_ = (0, 32) if half == 0 else (32, 32)
                mtp = ps.tile([C, 1024], F16, tag="sm", bufs=2, name=f"mtp{half}")
                Mcols = M8[:, lo:lo + 32, 7:8].rearrange("p a b -> p (a b)")
                nc.tensor.transpose(mtp[0:32, 0:128], Mcols, ident[:])
                nc.scalar.activation(Mt[lo:lo + 32, :], mtp[0:32, 0:128], AF.Copy)
                projn_r = projn_d.rearrange("(a b) -> a b", b=128)
                nc.sync.dma_start(projn_r[lo:lo + 32, :], Mt[lo:lo + 32, :])

            median_range(0, 8)
            median_range(8, 16)

            # ---------------- iter-1 g-MLP + conv h0-half (under median) -----
            def mlp_layer(w, h_in, out, it, lab, bias, alpha):
                for half, o0, on in ((0, 0, 1536), (1, 1536, HW - 1536)):
                    gp = ps.tile([C, 1536], F32, tag="big3", bufs=2,
                                 name=f"{lab}_{it}_{half}")
                    for c0, ncn in (CHUNKS6[:3] if half == 0 else CHUNKS6[3:]):
                        nc.tensor.matmul(gp[:, c0 - o0:c0 - o0 + ncn], w[:],
                                         h_in[:, c0:c0 + ncn], start=True, stop=True)
                    nc.scalar.activation(out[:, o0:o0 + on], gp[:, 0:on], AF.Prelu,
                                         bias=bias, alpha=alpha)

            def gmlp(h_in, it):
                gh1 = work.tile([C, 2720], F16, tag="gh", bufs=2, name=f"gh1_{it}")
                mlp_layer(gw0, h_in, gh1, it, "g1", bia[:, 0:1], a0)
                gh2 = work.tile([C, 2720], F16, tag="gh", bufs=2, name=f"gh2_{it}")
                mlp_layer(gw1, gh1, gh2, it, "g2", bia[:, 1:2], a1)
                return gh2

            def transposes(gh2, it):
                for grp, jts in enumerate(TGROUPS):
                    tp = ps.tile([C, 1024], F16, tag="sm", bufs=2,
                                 name=f"tp_{it}_{grp}")
                    for k, jt in enumerate(jts):
                        j0, nj = PTILES[jt]
                        nc.tensor.transpose(tp[0:nj, 128 * k:128 * k + 128],
                                            gh2[:, j0:j0 + nj], ident[:])
                    base = 1024 * grp
                    if grp < 2:
                        nc.scalar.activation(ghrm[:, base:base + 1024],
                                             tp[:, 0:1024], AF.Copy)
                    else:
                        nc.scalar.activation(ghrm[:, base:base + 640],
                                             tp[:, 0:640], AF.Copy)
                        nc.scalar.activation(ghrm[0:12, base + 640:base + 768],
                                             tp[0:12, 640:768], AF.Copy)

            gh2_1 = gmlp(h0, 0)
            transposes(gh2_1, 0)

            nc.scalar.activation(pad0[:, 1:H + 1, 1:W + 1],
                                 h0[:, 0:HW].rearrange("p (h w) -> p h w", h=H), AF.Copy)
            taps = [(a, b) for a in range(3) for b in range(3)]
            for ri, (r0, nr) in enumerate(RCHUNKS):
                cpe = ps.tile([C, 512], F32, tag="sm", bufs=2, name=f"cpe_{ri}")
                for ti, (dy, dx) in enumerate(taps):
                    idx = (dy * 3 + dx) * 2
                    nc.tensor.matmul(cpe[:, 0:nr * W], cw[:, idx, :],
                                     pad0[:, r0 + dy:r0 + dy + nr, dx:dx + W],
                                     start=(ti == 0), stop=(ti == 8))
                nc.scalar.activation(convacc[:, r0 * W:(r0 + nr) * W],
                                     cpe[:, 0:nr * W], AF.Identity, bias=bia[:, 3:4])

            # ---------------- proj flatten + U/V staging ---------------------
            flatten_half(0)
            # x channel DMA overlaps the second median half
            nc.sync.dma_start(V[0:1, 0:HW], projn_d[0:HW])
            sq3 = work.tile([3, 2720], F16, tag="sq3", bufs=1, name="sq3")

            flatten_half(1)
            for ch in (1, 2):
                nc.sync.dma_start(V[ch:ch + 1, 0:HW], projn_d[ch * HW:(ch + 1) * HW])
            nc.scalar.activation(U[0:3, 0:HW], V[0:3, 0:HW], AF.Copy, scale=2.0)
            nc.scalar.activation(sq3[0:3, 0:HW], V[0:3, 0:HW], AF.Square)
            sqp = ps.tile([C, 1536], F32, tag="big3", bufs=2, name="sqp")
            for c0, ncn in CHUNKS6[:3]:
                nc.tensor.matmul(sqp[0:1, c0:c0 + ncn], ones3[:],
                                 sq3[:, c0:c0 + ncn], start=True, stop=True)
            hirow = work.tile([1, 2816], F16, tag="row", bufs=2, name="hirow")
            nc.scalar.activation(hirow[0:1, 0:1536], sqp[0:1, 0:1536],
                                 AF.Copy, scale=-1.0)
            sqp2 = ps.tile([C, 1536], F32, tag="big3", bufs=2, name="sqp2")
            for c0, ncn in CHUNKS6[3:]:
                nc.tensor.matmul(sqp2[0:1, c0 - 1536:c0 - 1536 + ncn],
                                 ones3[:], sq3[:, c0:c0 + ncn], start=True, stop=True)
            nc.scalar.activation(hirow[0:1, 1536:HW], sqp2[0:1, 0:HW - 1536],
                                 AF.Copy, scale=-1.0)
            nc.sync.dma_start(V[3:4, 0:HW], hirow[0:1, 0:HW])

            # ---------------- p1: per-row te + sign, software-pipelined ------
            efs = {}

            def stage_ef(jt):
                i0, ni = PTILES[jt]
                ef = work.tile([C, 2720], F16, tag="ef", bufs=3, name=f"ef_{jt}")
                efs[jt] = ef
                for half, o0, on in ((0, 0, 1536), (1, 1536, HW - 1536)):
                    pp = ps.tile([C, 1536], F32, tag="big3", bufs=2,
                                 name=f"pp_{jt}_{half}")
                    for c0, ncn in (CHUNKS6[:3] if half == 0 else CHUNKS6[3:]):
                        nc.tensor.matmul(pp[0:ni, c0 - o0:c0 - o0 + ncn],
                                         U[0:4, i0:i0 + ni], V[0:4, c0:c0 + ncn],
                                         start=True, stop=True)
                    nc.scalar.activation(ef[0:ni, o0:o0 + on], pp[0:ni, 0:on],
                                         AF.Copy)

            def p1_scan(jt):
                i0, ni = PTILES[jt]
                ef = efs[jt]
                t8a = work.tile([C, 8], F16, tag="t8", bufs=4, name=f"t8a_{jt}")
                nc.vector.max(t8a[0:ni], ef[0:ni, 0:HW])
                v8f = work.tile([C, 1], F32, tag="v8f", bufs=8, name=f"v8f_{jt}")
                nc.vector.tensor_copy(v8f[0:ni], t8a[0:ni, 7:8])
                msk = work.tile([C, 2720], F16, tag="msk", bufs=2, name=f"msk_{jt}")
                nc.vector.tensor_scalar(msk[0:ni, 0:HW], ef[0:ni, 0:HW],
                                        v8f[0:ni], NEG_F16,
                                        op0=ALU.is_ge, op1=ALU.mult)
                eft = work.tile([C, 2720], F16, tag="msk", bufs=2, name=f"eft_{jt}")
                nc.vector.tensor_tensor(eft[0:ni, 0:HW], ef[0:ni, 0:HW],
                                        msk[0:ni, 0:HW], ALU.add)
                t8b = work.tile([C, 8], F16, tag="t8", bufs=4, name=f"t8b_{jt}")
                nc.vector.max(t8b[0:ni], eft[0:ni, 0:HW])
                # bias = -te + |te|*2^-11 + 4e-7
                tp1 = work.tile([C, 1], F32, tag="v8f", bufs=8, name=f"tp1_{jt}")
                nc.vector.tensor_scalar(tp1[0:ni], t8b[0:ni, 7:8], 2.0 ** -11, 0.0,
                                        op0=ALU.mult, op1=ALU.add)
                tab = work.tile([C, 1], F32, tag="v8f", bufs=8, name=f"tab_{jt}")
                nc.vector.scalar_tensor_tensor(tab[0:ni], t8b[0:ni, 7:8],
                                               -(2.0 ** -11), tp1[0:ni],
                                               ALU.mult, ALU.max)
                bv = work.tile([C, 1], F32, tag="v8f", bufs=8, name=f"bv_{jt}")
                nc.vector.scalar_tensor_tensor(bv[0:ni], tab[0:ni], 4.0e-7,
                                               t8b[0:ni, 7:8], ALU.add, ALU.subtract)
                return bv

            def p1_sign(jt, bv):
                i0, ni = PTILES[jt]
                nc.scalar.activation(Sbig[0:ni, jt, 0:HW], efs[jt][0:ni, 0:HW],
                                     AF.Sign, bias=bv[0:ni])

            stage_ef(0)
            stage_ef(1)
            for jt in range(22):
                bv = p1_scan(jt)
                if jt + 2 < 22:
                    stage_ef(jt + 2)
                p1_sign(jt, bv)

            # ---------------- dense tail: agg1, q1, gmlp2, agg2, q2, conv ----
            def agg_block(it):
                A = ps.tile([C, 1536], F32, tag="big3", bufs=2, name=f"agg{it}A")
                B = ps.tile([C, 1536], F32, tag="big3", bufs=2, name=f"agg{it}B")

                def tgt(c0, ncn):
                    return A[:, c0:c0 + ncn] if c0 < 1536 else B[:, c0 - 1536:c0 - 1536 + ncn]

                for jt, (j0, nj) in enumerate(PTILES):
                    for c0, ncn in AGG_CHUNKS:
                        nc.tensor.matmul(tgt(c0, ncn),
                                         ghrm[0:nj, 128 * jt:128 * jt + 128],
                                         Sbig[0:nj, jt, c0:c0 + ncn],
                                         start=(jt == 0), stop=False)
                # G broadcast: extract G col, transpose to a row, rank-1 add
                gcol = sb.tile([C, 1], F16, tag=f"gcol_{it}")
                nc.scalar.activation(gcol[:], B[:, 1164:1165], AF.Copy)
                gpt = ps.tile([C, 1024], F16, tag="sm", bufs=2, name=f"gpt_{it}")
                nc.tensor.transpose(gpt[0:1, 0:128], gcol[:, 0:1], ident[:])
                grow = sb.tile([1, C], F16, tag=f"grow_{it}")
                nc.scalar.activation(grow[:], gpt[0:1, 0:128], AF.Copy)
                for c0, ncn in AGG_CHUNKS:
                    nc.tensor.matmul(tgt(c0, ncn), grow[0:1, 0:C],
                                     onesrow[0:1, c0:c0 + ncn],
                                     start=False, stop=True)
                mts = work.tile([C, 2720], F16, tag="mts", bufs=1, name=f"mts_{it}")
                nc.scalar.activation(mts[:, 0:1536], A[:, 0:1536], AF.Copy)
                nc.scalar.activation(mts[:, 1536:HW], B[:, 0:1164], AF.Copy)
                return mts

            def q_update(h_in, mts, it):
                h_out = work.tile([C, 2720], F16, tag="h", bufs=2, name=f"h_{it}")
                for half, o0, on in ((0, 0, 1536), (1, 1536, HW - 1536)):
                    qp = ps.tile([C, 1536], F32, tag="big3", bufs=2,
                                 name=f"qp_{it}_{half}")
                    for c0, ncn in (CHUNKS6[:3] if half == 0 else CHUNKS6[3:]):
                        nc.tensor.matmul(qp[:, c0 - o0:c0 - o0 + ncn], qw1[:],
                                         h_in[:, c0:c0 + ncn], start=True, stop=False)
                        nc.tensor.matmul(qp[:, c0 - o0:c0 - o0 + ncn], qw2[:],
                                         mts[:, c0:c0 + ncn], start=False, stop=True)
                    nc.scalar.activation(h_out[:, o0:o0 + on], qp[:, 0:on], AF.Prelu,
                                         bias=bia[:, 2:3], alpha=qa)
                return h_out

            mts1 = agg_block(0)
            h1 = q_update(h0, mts1, 0)
            gh2_2 = gmlp(h1, 1)
            transposes(gh2_2, 1)
            mts2 = agg_block(1)
            h2 = q_update(h1, mts2, 1)

            # conv h2-half
            nc.scalar.activation(pad1[:, 1:H + 1, 1:W + 1],
                                 h2[:, 0:HW].rearrange("p (h w) -> p h w", h=H), AF.Copy)
            oc = work.tile([C, 2720], F32, tag="bigf32", bufs=1, name="oc")
            for ri, (r0, nr) in enumerate(RCHUNKS):
                cpe = ps.tile([C, 512], F32, tag="sm", bufs=2, name=f"cp2_{ri}")
                for ti, (dy, dx) in enumerate(taps):
                    idx = (dy * 3 + dx) * 2 + 1
                    nc.tensor.matmul(cpe[:, 0:nr * W], cw[:, idx, :],
                                     pad1[:, r0 + dy:r0 + dy + nr, dx:dx + W],
                                     start=(ti == 0), stop=(ti == 8))
                nc.vector.tensor_tensor(oc[:, r0 * W:(r0 + nr) * W],
                                        cpe[:, 0:nr * W],
                                        convacc[:, r0 * W:(r0 + nr) * W], ALU.add)
                if ri == 2:
                    nc.sync.dma_start(out_d[:, 0:1440], oc[:, 0:1440])
                elif ri == 5:
                    nc.sync.dma_start(out_d[:, 1440:2700], oc[:, 1440:2700])

    nc.compile()
    return nc


def kernel(cnn_encoder_output, original_input, xy,
           g_w0, g_b0, g_a0, g_w1, g_b1, g_a1,
           q_w, q_b, q_a, conv_w, conv_b,
           gnn_iterations, k, use_half_precision, _trace=False):
    assert int(gnn_iterations) == 2 and int(k) == 16 and int(use_half_precision) == 0

    cnn = np.asarray(cnn_encoder_output, dtype=np.float32)
    orig = np.asarray(original_input, dtype=np.float32)
    xy = np.asarray(xy, dtype=np.float32)
    a0, a1, qa = float(np.ravel(g_a0)[0]), float(np.ravel(g_a1)[0]), float(np.ravel(q_a)[0])

    key = (a0, a1, qa)
    if key not in _cache:
        _cache[key] = _build(a0, a1, qa)
    nc = _cache[key]

    g_w0 = np.asarray(g_w0, np.float32)
    g_w1 = np.asarray(g_w1, np.float32)
    q_w = np.asarray(q_w, np.float32)
    conv_w = np.asarray(conv_w, np.float32)

    gw0T = np.ascontiguousarray(g_w0.T).astype(np.float16)
    gw1T = np.ascontiguousarray(g_w1.T).astype(np.float16)
    qw1T = np.ascontiguousarray(q_w[:, :C].T).astype(np.float16)
    qw2T = np.ascontiguousarray(q_w[:, C:].T / float(2 * K)).astype(np.float16)
    cwT = np.empty((C, 18, C), np.float16)
    for dy in range(3):
        for dx in range(3):
            for kh in range(2):
                idx = (dy * 3 + dx) * 2 + kh
                cwT[:, idx, :] = conv_w[:, kh * C:(kh + 1) * C, dy, dx].T.astype(np.float16)
    biases = np.stack([np.asarray(g_b0, np.float32), np.asarray(g_b1, np.float32),
                       np.asarray(q_b, np.float32), np.asarray(conv_b, np.float32)],
                      axis=1)
    ident = np.eye(C, dtype=np.float16)
    uvc = np.zeros((2, 8, 2816), np.float16)
    uvc[0, 3] = 1.0

    shared = dict(gw0T=gw0T, gw1T=gw1T, qw1T=qw1T, qw2T=qw2T, convwT=cwT,
                  biases=np.ascontiguousarray(biases), ident=ident, uvc=uvc)
    in_maps = []
    for n in range(N):
        chans = np.stack([xy[n, 0], xy[n, 1], orig[n, 3]], axis=0)      # [3, 360, 480]
        blocks = chans.reshape(3, H, 8, W, 8).transpose(0, 1, 3, 2, 4).reshape(3 * HW, 64)
        blocks = (-blocks).astype(np.float16)
        pad = np.zeros((8192, 64), np.float16)
        pad[:3 * HW] = blocks
        psrcb = pad.reshape(16, 4, 128, 64).transpose(0, 2, 1, 3)
        in_maps.append(dict(h0=np.ascontiguousarray(
                                cnn[n].reshape(C, HW).astype(np.float16)),
                            psrcb=np.ascontiguousarray(psrcb), **shared))

    if _trace:
        _ensure_ntff_hook()
    res = run_bass_kernel_spmd(nc, in_maps, core_ids=list(range(N)), trace=_trace,
                               trace_cores=list(range(N)) if _trace else None)
    out = np.stack([res.results[n]["out"].reshape(C, H, W).astype(np.float32)
                    for n in range(N)])
    if _trace:
        kernel._last_results = res
    return out


# revision 19
# speedup vs baseline: 1.3115x; 1.0248x over previous
"""Trainium2 Bass kernel for EnetGnn (gnn_message_passing).

Data-parallel over batch N=8, one sample per NeuronCore. Per-core design:

1. Median pool: host stages negated fp16 blocks in [16, 128, 4, 64] tiles so
   each load is one contiguous 64KB DMA. DVE max8/match_replace rank-32
   rounds; medians flattened via two half PE transposes + DMA so the x
   channel stages while the second half of the median still runs.
2. KNN threshold: e'[i,j] = 2p_i.p_j - |p_j|^2 via K=4 fp16 matmuls into
   double-buffered 3-bank psum halves, ACT-evacuated to fp16 ef. Per-row
   16th-largest te via max8 + is_ge mask removal + max8 (all DVE).
3. S = Sign(ef - te + eps) on the ACT engine with per-row bias (no phase-2
   matmul recompute). S tiles land in one [128, 22, 2720] fp8 SBUF tensor
   with a ones-column for G.
4. All heavy matmul work (aggregation iters 1+2, q updates, g-MLP 2, conv)
   runs as one dense tail stream to keep the PE at its hot clock. The
   ones-column gives G; a rank-1 matmul broadcasts G so mts = G + S@gh
   evacuates at natural scale to fp16, making the q update all-fp16.
"""
import numpy as np
import concourse.bass as bass
import concourse.bacc as bacc
import concourse.mybir as mybir
import concourse.tile as tile
from concourse.bass_utils import run_bass_kernel_spmd

F32 = mybir.dt.float32
F16 = mybir.dt.float16
F8 = mybir.dt.float8e4
AF = mybir.ActivationFunctionType
ALU = mybir.AluOpType

N, C, H, W = 8, 128, 45, 60
HW = H * W                      # 2700
K = 16
NEG_F16 = -60000.0

CHUNKS6 = [(0, 512), (512, 512), (1024, 512), (1536, 512), (2048, 512), (2560, 140)]
AGG_CHUNKS = [(0, 512), (512, 512), (1024, 512), (1536, 512), (2048, 512), (2560, 141)]
PTILES = [(t * 128, 128) for t in range(21)] + [(2688, 12)]
RCHUNKS = [(0, 8), (8, 8), (16, 8), (24, 8), (32, 8), (40, 5)]
TGROUPS = [list(range(0, 8)), list(range(8, 16)), list(range(16, 22))]

_cache = {}


def _ensure_ntff_hook():
    import sys
    import types
    try:
        from antenv.axon_hooks import get_axon_ntff_profile_hook  # noqa: F401
        return
    except ImportError:
        pass
    try:
        mod = types.ModuleType("antenv.axon_hooks")
        mod._hook = None

        def set_axon_ntff_profile_hook(h):
            mod._hook = h

        def get_axon_ntff_profile_hook():
            return mod._hook

        mod.set_axon_ntff_profile_hook = set_axon_ntff_profile_hook
        mod.get_axon_ntff_profile_hook = get_axon_ntff_profile_hook
        sys.modules["antenv.axon_hooks"] = mod
        import antenv
        antenv.axon_hooks = mod
        from trn_agent_boot.trn_boot import _ntff_profile_via_ctypes
        hook = _ntff_profile_via_ctypes("/opt/axon/libaxon_pjrt.so")
        if hook is not None:
            mod.set_axon_ntff_profile_hook(hook)
    except Exception as e:  # profiling is best-effort
        print(f"ntff hook injection failed: {e}")


def _build(a0, a1, qa):
    nc = bacc.Bacc("TRN2", target_bir_lowering=False, debug=False, num_devices=8)

    h0_d = nc.dram_tensor("h0", (C, HW), F16, kind="ExternalInput")
    psrcb_d = nc.dram_tensor("psrcb", (16, 128, 4, 64), F16, kind="ExternalInput")
    gw0_d = nc.dram_tensor("gw0T", (C, C), F16, kind="ExternalInput")
    gw1_d = nc.dram_tensor("gw1T", (C, C), F16, kind="ExternalInput")
    qw1_d = nc.dram_tensor("qw1T", (C, C), F16, kind="ExternalInput")
    qw2_d = nc.dram_tensor("qw2T", (C, C), F16, kind="ExternalInput")
    cw_d = nc.dram_tensor("convwT", (C, 18, C), F16, kind="ExternalInput")
    bias_d = nc.dram_tensor("biases", (C, 4), F32, kind="ExternalInput")
    ident_d = nc.dram_tensor("ident", (C, C), F16, kind="ExternalInput")
    uvc_d = nc.dram_tensor("uvc", (2, 8, 2816), F16, kind="ExternalInput")
    out_d = nc.dram_tensor("out", (C, HW), F32, kind="ExternalOutput")

    with tile.TileContext(nc) as tc:
        with tc.tile_pool(name="sb", bufs=1) as sb, \
             tc.tile_pool(name="work", bufs=2) as work, \
             tc.tile_pool(name="ps", bufs=1, space="PSUM") as ps, \
             tc.tile_pool(name="dram", bufs=1, space="DRAM") as dram:

            projn_d = dram.tile([8192], F16, tag="projn_d")

            # median block DMAs first so the DVE phase starts immediately
            blks = []
            for g in range(16):
                blk = work.tile([128, 4, 64], F16, tag="blk", bufs=8,
                                name=f"blk_{g}")
                nc.sync.dma_start(blk[:], psrcb_d[g])
                blks.append(blk)

            # ---------------- persistent SBUF ----------------
            h0 = sb.tile([C, 2720], F16, tag="h0")
            nc.sync.dma_start(h0[:, 0:HW], h0_d[:])
            gw0 = sb.tile([C, C], F16, tag="gw0")
            nc.sync.dma_start(gw0[:], gw0_d[:])
            gw1 = sb.tile([C, C], F16, tag="gw1")
            nc.sync.dma_start(gw1[:], gw1_d[:])
            qw1 = sb.tile([C, C], F16, tag="qw1")
            nc.sync.dma_start(qw1[:], qw1_d[:])
            qw2 = sb.tile([C, C], F16, tag="qw2")
            nc.sync.dma_start(qw2[:], qw2_d[:])
            cw = sb.tile([C, 18, C], F16, tag="cw")
            nc.sync.dma_start(cw[:], cw_d[:])
            bia = sb.tile([C, 4], F32, tag="bias")
            nc.sync.dma_start(bia[:], bias_d[:])
            ident = sb.tile([C, C], F16, tag="ident")
            nc.sync.dma_start(ident[:], ident_d[:])

            U = sb.tile([8, 2816], F16, tag="U")       # [q; 1]
            nc.sync.dma_start(U[:], uvc_d[0])
            V = sb.tile([8, 2816], F16, tag="V")       # [q; -|p|^2/2]
            nc.sync.dma_start(V[:], uvc_d[1])
            Sbig = sb.tile([C, 22, 2720], F8, tag="Sbig")
            ghrm8 = sb.tile([C, 22, 128], F8, tag="ghrm8")
            M8 = sb.tile([C, 64, 8], F16, tag="M8")
            Mt = sb.tile([64, C], F16, tag="Mt")
            pad0 = sb.tile([C, H + 2, W + 2], F16, tag="pad0")
            pad1 = sb.tile([C, H + 2, W + 2], F16, tag="pad1")
            convacc = sb.tile([C, 2720], F32, tag="convacc")
            ones3 = sb.tile([3, 1], F16, tag="ones3")

            # memsets on gpsimd (DVE stays on the median path)
            nc.gpsimd.memset(Sbig[:, :, HW:HW + 1], 1.0)   # ones-cols for G
            nc.gpsimd.memset(pad0[:], 0.0)
            nc.gpsimd.memset(pad1[:], 0.0)
            nc.gpsimd.memset(ones3[:], 1.0)

            # ---------------- median pooling + split flatten -----------------
            def median_range(glo, ghi):
                for g in range(glo, ghi):
                    blk = blks[g]
                    for s in range(4):
                        mm8 = work.tile([128, 8], F16, tag="mm8", bufs=8)
                        for rnd in range(3):
                            nc.vector.max(mm8[:], blk[:, s, :])
                            nc.vector.match_replace(blk[:, s, :], mm8[:],
                                                    blk[:, s, :], NEG_F16)
                        nc.vector.max(M8[:, g * 4 + s, :], blk[:, s, :])

            def flatten_half(half):
                lo, nc_ = (0, 32) if half == 0 else (32, 32)
                mtp = ps.tile([C, 1024], F16, tag="sm", bufs=2, name=f"mtp{half}")
                Mcols = M8[:, lo:lo + 32, 7:8].rearrange("p a b -> p (a b)")
                nc.tensor.transpose(mtp[0:32, 0:128], Mcols, ident[:])
                nc.scalar.activation(Mt[lo:lo + 32, :], mtp[0:32, 0:128], AF.Copy)
                projn_r = projn_d.rearrange("(a b) -> a b", b=128)
                nc.sync.dma_start(projn_r[lo:lo + 32, :], Mt[lo:lo + 32, :])

            median_range(0, 8)
            median_range(8, 16)

            # ---------------- iter-1 g-MLP + conv h0-half (under median) -----
            def mlp_layer(w, h_in, out, it, lab, bias, alpha):
                for half, o0, on in ((0, 0, 1536), (1, 1536, HW - 1536)):
                    gp = ps.tile([C, 1536], F32, tag="big3", bufs=2,
                                 name=f"{lab}_{it}_{half}")
                    for c0, ncn in (CHUNKS6[:3] if half == 0 else CHUNKS6[3:]):
                        nc.tensor.matmul(gp[:, c0 - o0:c0 - o0 + ncn], w[:],
                                         h_in[:, c0:c0 + ncn], start=True, stop=True)
                    nc.scalar.activation(out[:, o0:o0 + on], gp[:, 0:on], AF.Prelu,
                                         bias=bias, alpha=alpha)

            def gmlp(h_in, it):
                gh1 = work.tile([C, 2720], F16, tag="gh", bufs=2, name=f"gh1_{it}")
                mlp_layer(gw0, h_in, gh1, it, "g1", bia[:, 0:1], a0)
                gh2 = work.tile([C, 2720], F16, tag="gh", bufs=2, name=f"gh2_{it}")
                mlp_layer(gw1, gh1, gh2, it, "g2", bia[:, 1:2], a1)
                return gh2

            def transposes(gh2, it):
                for grp, jts in enumerate(TGROUPS):
                    tp = ps.tile([C, 1024], F16, tag="sm", bufs=2,
                                 name=f"tp_{it}_{grp}")
                    for k, jt in enumerate(jts):
                        j0, nj = PTILES[jt]
                        nc.tensor.transpose(tp[0:nj, 128 * k:128 * k + 128],
                                            gh2[:, j0:j0 + nj], ident[:])
                    t0 = grp * 8
                    if grp < 2:
                        nc.scalar.activation(
                            ghrm8[:, t0:t0 + 8, :],
                            tp[:, 0:1024].rearrange("p (a b) -> p a b", b=128),
                            AF.Copy)
                    else:
                        nc.scalar.activation(
                            ghrm8[:, t0:t0 + 5, :],
                            tp[:, 0:640].rearrange("p (a b) -> p a b", b=128),
                            AF.Copy)
                        nc.scalar.activation(
                            ghrm8[0:12, 21, :],
                            tp[0:12, 640:768], AF.Copy)

            gh2_1 = gmlp(h0, 0)
            transposes(gh2_1, 0)

            nc.scalar.activation(pad0[:, 1:H + 1, 1:W + 1],
                                 h0[:, 0:HW].rearrange("p (h w) -> p h w", h=H), AF.Copy)
            taps = [(a, b) for a in range(3) for b in range(3)]
            for ri, (r0, nr) in enumerate(RCHUNKS):
                cpe = ps.tile([C, 512], F32, tag="sm", bufs=2, name=f"cpe_{ri}")
                for ti, (dy, dx) in enumerate(taps):
                    idx = (dy * 3 + dx) * 2
                    nc.tensor.matmul(cpe[:, 0:nr * W], cw[:, idx, :],
                                     pad0[:, r0 + dy:r0 + dy + nr, dx:dx + W],
                                     start=(ti == 0), stop=(ti == 8))
                nc.scalar.activation(convacc[:, r0 * W:(r0 + nr) * W],
                                     cpe[:, 0:nr * W], AF.Identity, bias=bia[:, 3:4])

            # ---------------- proj flatten + U/V staging ---------------------
            flatten_half(0)
            # x channel DMA overlaps the second median half
            nc.sync.dma_start(V[0:1, 0:HW], projn_d[0:HW])
            sq3 = work.tile([3, 2720], F16, tag="sq3", bufs=1, name="sq3")

            nc.sync.dma_start(U[0:1, 0:HW], projn_d[0:HW])
            flatten_half(1)
            for ch in (1, 2):
                nc.sync.dma_start(V[ch:ch + 1, 0:HW], projn_d[ch * HW:(ch + 1) * HW])
                nc.sync.dma_start(U[ch:ch + 1, 0:HW], projn_d[ch * HW:(ch + 1) * HW])
            # e'' = q.p - |p_j|^2/2 (same order as e' = 2q.p - |p_j|^2)
            nc.vector.tensor_tensor(sq3[0:3, 0:HW], V[0:3, 0:HW], V[0:3, 0:HW],
                                    ALU.mult)
            sqp = ps.tile([C, 1536], F32, tag="big3", bufs=2, name="sqp")
            for c0, ncn in CHUNKS6[:3]:
                nc.tensor.matmul(sqp[0:1, c0:c0 + ncn], ones3[:],
                                 sq3[:, c0:c0 + ncn], start=True, stop=True)
            hirow = work.tile([1, 2816], F16, tag="row", bufs=2, name="hirow")
            nc.scalar.activation(hirow[0:1, 0:1536], sqp[0:1, 0:1536],
                                 AF.Copy, scale=-0.5)
            sqp2 = ps.tile([C, 1536], F32, tag="big3", bufs=2, name="sqp2")
            for c0, ncn in CHUNKS6[3:]:
                nc.tensor.matmul(sqp2[0:1, c0 - 1536:c0 - 1536 + ncn],
                                 ones3[:], sq3[:, c0:c0 + ncn], start=True, stop=True)
            nc.scalar.activation(hirow[0:1, 1536:HW], sqp2[0:1, 0:HW - 1536],
                                 AF.Copy, scale=-0.5)
            nc.sync.dma_start(V[3:4, 0:HW], hirow[0:1, 0:HW])

            # ---------------- p1: per-row te + sign, software-pipelined ------
            efs = {}

            def stage_ef(jt):
                i0, ni = PTILES[jt]
                ef = work.tile([C, 2720], F16, tag="ef", bufs=3, name=f"ef_{jt}")
                efs[jt] = ef
                for half, o0, on in ((0, 0, 1536), (1, 1536, HW - 1536)):
                    pp = ps.tile([C, 1536], F32, tag="big3", bufs=2,
                                 name=f"pp_{jt}_{half}")
                    for c0, ncn in (CHUNKS6[:3] if half == 0 else CHUNKS6[3:]):
                        nc.tensor.matmul(pp[0:ni, c0 - o0:c0 - o0 + ncn],
                                         U[0:4, i0:i0 + ni], V[0:4, c0:c0 + ncn],
                                         start=True, stop=True)
                    nc.scalar.activation(ef[0:ni, o0:o0 + on], pp[0:ni, 0:on],
                                         AF.Copy)

            def p1_scan(jt):
                i0, ni = PTILES[jt]
                ef = efs[jt]
                t8a = work.tile([C, 8], F16, tag="t8", bufs=4, name=f"t8a_{jt}")
                nc.vector.max(t8a[0:ni], ef[0:ni, 0:HW])
                v8f = work.tile([C, 1], F32, tag="v8f", bufs=8, name=f"v8f_{jt}")
                nc.vector.tensor_copy(v8f[0:ni], t8a[0:ni, 7:8])
                msk = work.tile([C, 2720], F16, tag="msk", bufs=2, name=f"msk_{jt}")
                nc.vector.tensor_scalar(msk[0:ni, 0:HW], ef[0:ni, 0:HW],
                                        v8f[0:ni], NEG_F16,
                                        op0=ALU.is_ge, op1=ALU.mult)
                eft = work.tile([C, 2720], F16, tag="msk", bufs=2, name=f"eft_{jt}")
                nc.vector.tensor_tensor(eft[0:ni, 0:HW], ef[0:ni, 0:HW],
                                        msk[0:ni, 0:HW], ALU.add)
                t8b = work.tile([C, 8], F16, tag="t8", bufs=4, name=f"t8b_{jt}")
                nc.vector.max(t8b[0:ni], eft[0:ni, 0:HW])
                # bias = -te + |te|*2^-11 + 4e-7
                tp1 = work.tile([C, 1], F32, tag="v8f", bufs=8, name=f"tp1_{jt}")
                nc.vector.tensor_scalar(tp1[0:ni], t8b[0:ni, 7:8], 2.0 ** -11, 0.0,
                                        op0=ALU.mult, op1=ALU.add)
                tab = work.tile([C, 1], F32, tag="v8f", bufs=8, name=f"tab_{jt}")
                nc.vector.scalar_tensor_tensor(tab[0:ni], t8b[0:ni, 7:8],
                                               -(2.0 ** -11), tp1[0:ni],
                                               ALU.mult, ALU.max)
                bv = work.tile([C, 1], F32, tag="v8f", bufs=8, name=f"bv_{jt}")
                nc.vector.scalar_tensor_tensor(bv[0:ni], tab[0:ni], 4.0e-7,
                                               t8b[0:ni, 7:8], ALU.add, ALU.subtract)
                return bv

            def p1_sign(jt, bv):
                i0, ni = PTILES[jt]
                nc.scalar.activation(Sbig[0:ni, jt, 0:HW], efs[jt][0:ni, 0:HW],
                                     AF.Sign, bias=bv[0:ni])

            stage_ef(0)
            stage_ef(1)
            for jt in range(22):
                bv = p1_scan(jt)
                if jt + 2 < 22:
                    stage_ef(jt + 2)
                p1_sign(jt, bv)

            # ---------------- dense tail: agg1, q1, gmlp2, agg2, q2, conv ----
            DR = mybir.MatmulPerfMode.DoubleRow

            def agg_block(it):
                A = ps.tile([C, 1536], F32, tag="big3", bufs=2, name=f"agg{it}A")
                B = ps.tile([C, 1536], F32, tag="big3", bufs=2, name=f"agg{it}B")

                def tgt(c0, ncn):
                    return A[:, c0:c0 + ncn] if c0 < 1536 else B[:, c0 - 1536:c0 - 1536 + ncn]

                # pairs of full 128-row tiles via fp8 DoubleRow (2 k-tiles per
                # matmul), then tiles 20 (128 rows) and 21 (12 rows) normally
                for c0, ncn in AGG_CHUNKS:
                    for pr in range(10):
                        nc.tensor.matmul(tgt(c0, ncn),
                                         ghrm8[:, 2 * pr:2 * pr + 2, :],
                                         Sbig[:, 2 * pr:2 * pr + 2, c0:c0 + ncn],
                                         start=(pr == 0), stop=False,
                                         perf_mode=DR)
                    for jt in (20, 21):
                        j0, nj = PTILES[jt]
                        nc.tensor.matmul(tgt(c0, ncn),
                                         ghrm8[0:nj, jt, :],
                                         Sbig[0:nj, jt, c0:c0 + ncn],
                                         start=False, stop=(jt == 21))
                # G (ones-column row sums) folded in as per-partition bias
                gcol = sb.tile([C, 1], F32, tag=f"gcol_{it}")
                nc.scalar.activation(gcol[:], B[:, 1164:1165], AF.Copy)
                mts = work.tile([C, 2720], F16, tag="mts", bufs=1, name=f"mts_{it}")
                nc.scalar.activation(mts[:, 0:1536], A[:, 0:1536], AF.Identity,
                                     bias=gcol[:])
                nc.scalar.activation(mts[:, 1536:HW], B[:, 0:1164], AF.Identity,
                                     bias=gcol[:])
                return mts

            def q_update(h_in, mts, it):
                h_out = work.tile([C, 2720], F16, tag="h", bufs=2, name=f"h_{it}")
                for half, o0, on in ((0, 0, 1536), (1, 1536, HW - 1536)):
                    qp = ps.tile([C, 1536], F32, tag="big3", bufs=2,
                                 name=f"qp_{it}_{half}")
                    for c0, ncn in (CHUNKS6[:3] if half == 0 else CHUNKS6[3:]):
                        nc.tensor.matmul(qp[:, c0 - o0:c0 - o0 + ncn], qw1[:],
                                         h_in[:, c0:c0 + ncn], start=True, stop=False)
                        nc.tensor.matmul(qp[:, c0 - o0:c0 - o0 + ncn], qw2[:],
                                         mts[:, c0:c0 + ncn], start=False, stop=True)
                    nc.scalar.activation(h_out[:, o0:o0 + on], qp[:, 0:on], AF.Prelu,
                                         bias=bia[:, 2:3], alpha=qa)
                return h_out

            mts1 = agg_block(0)
            h1 = q_update(h0, mts1, 0)
            gh2_2 = gmlp(h1, 1)
            transposes(gh2_2, 1)
            mts2 = agg_block(1)

            # q2 with pad1 copies interleaved after each prelu half so the
            # conv matmuls never stall on the pad copy
            h2 = work.tile([C, 2720], F16, tag="h", bufs=2, name="h_1")
            for half, o0, on in ((0, 0, 1536), (1, 1536, HW - 1536)):
                qp = ps.tile([C, 1536], F32, tag="big3", bufs=2,
                             name=f"qp_1_{half}")
                for c0, ncn in (CHUNKS6[:3] if half == 0 else CHUNKS6[3:]):
                    nc.tensor.matmul(qp[:, c0 - o0:c0 - o0 + ncn], qw1[:],
                                     h1[:, c0:c0 + ncn], start=True, stop=False)
                    nc.tensor.matmul(qp[:, c0 - o0:c0 - o0 + ncn], qw2[:],
                                     mts2[:, c0:c0 + ncn], start=False, stop=True)
                nc.scalar.activation(h2[:, o0:o0 + on], qp[:, 0:on], AF.Prelu,
                                     bias=bia[:, 2:3], alpha=qa)
                if half == 0:
                    nc.scalar.activation(
                        pad1[:, 1:26, 1:W + 1],
                        h2[:, 0:1500].rearrange("p (h w) -> p h w", w=W), AF.Copy)
                else:
                    nc.scalar.activation(
                        pad1[:, 26:H + 1, 1:W + 1],
                        h2[:, 1500:HW].rearrange("p (h w) -> p h w", w=W), AF.Copy)

            oc = work.tile([C, 2720], F32, tag="bigf32", bufs=1, name="oc")
            for ri, (r0, nr) in enumerate(RCHUNKS):
                cpe = ps.tile([C, 512], F32, tag="sm", bufs=2, name=f"cp2_{ri}")
                for ti, (dy, dx) in enumerate(taps):
                    idx = (dy * 3 + dx) * 2 + 1
                    nc.tensor.matmul(cpe[:, 0:nr * W], cw[:, idx, :],
                                     pad1[:, r0 + dy:r0 + dy + nr, dx:dx + W],
                                     start=(ti == 0), stop=(ti == 8))
                nc.vector.tensor_tensor(oc[:, r0 * W:(r0 + nr) * W],
                                        cpe[:, 0:nr * W],
                                        convacc[:, r0 * W:(r0 + nr) * W], ALU.add)
                if ri == 2:
                    nc.sync.dma_start(out_d[:, 0:1440], oc[:, 0:1440])
                elif ri == 5:
                    nc.sync.dma_start(out_d[:, 1440:2700], oc[:, 1440:2700])

    nc.compile()
    return nc


def kernel(cnn_encoder_output, original_input, xy,
           g_w0, g_b0, g_a0, g_w1, g_b1, g_a1,
           q_w, q_b, q_a, conv_w, conv_b,
           gnn_iterations, k, use_half_precision, _trace=False):
    assert int(gnn_iterations) == 2 and int(k) == 16 and int(use_half_precision) == 0

    cnn = np.asarray(cnn_encoder_output, dtype=np.float32)
    orig = np.asarray(original_input, dtype=np.float32)
    xy = np.asarray(xy, dtype=np.float32)
    a0, a1, qa = float(np.ravel(g_a0)[0]), float(np.ravel(g_a1)[0]), float(np.ravel(q_a)[0])

    key = (a0, a1, qa)
    if key not in _cache:
        _cache[key] = _build(a0, a1, qa)
    nc = _cache[key]

    g_w0 = np.asarray(g_w0, np.float32)
    g_w1 = np.asarray(g_w1, np.float32)
    q_w = np.asarray(q_w, np.float32)
    conv_w = np.asarray(conv_w, np.float32)

    gw0T = np.ascontiguousarray(g_w0.T).astype(np.float16)
    gw1T = np.ascontiguousarray(g_w1.T).astype(np.float16)
    qw1T = np.ascontiguousarray(q_w[:, :C].T).astype(np.float16)
    qw2T = np.ascontiguousarray(q_w[:, C:].T / float(2 * K)).astype(np.float16)
    cwT = np.empty((C, 18, C), np.float16)
    for dy in range(3):
        for dx in range(3):
            for kh in range(2):
                idx = (dy * 3 + dx) * 2 + kh
                cwT[:, idx, :] = conv_w[:, kh * C:(kh + 1) * C, dy, dx].T.astype(np.float16)
    biases = np.stack([np.asarray(g_b0, np.float32), np.asarray(g_b1, np.float32),
                       np.asarray(q_b, np.float32), np.asarray(conv_b, np.float32)],
                      axis=1)
    ident = np.eye(C, dtype=np.float16)
    uvc = np.zeros((2, 8, 2816), np.float16)
    uvc[0, 3] = 1.0

    shared = dict(gw0T=gw0T, gw1T=gw1T, qw1T=qw1T, qw2T=qw2T, convwT=cwT,
                  biases=np.ascontiguousarray(biases), ident=ident, uvc=uvc)
    in_maps = []
    for n in range(N):
        chans = np.stack([xy[n, 0], xy[n, 1], orig[n, 3]], axis=0)      # [3, 360, 480]
        blocks = chans.reshape(3, H, 8, W, 8).transpose(0, 1, 3, 2, 4).reshape(3 * HW, 64)
        blocks = (-blocks).astype(np.float16)
        pad = np.zeros((8192, 64), np.float16)
        pad[:3 * HW] = blocks
        psrcb = pad.reshape(16, 4, 128, 64).transpose(0, 2, 1, 3)
        in_maps.append(dict(h0=np.ascontiguousarray(
                                cnn[n].reshape(C, HW).astype(np.float16)),
                            psrcb=np.ascontiguousarray(psrcb), **shared))

    if _trace:
        _ensure_ntff_hook()
    res = run_bass_kernel_spmd(nc, in_maps, core_ids=list(range(N)), trace=_trace,
                               trace_cores=list(range(N)) if _trace else None)
    out = np.stack([res.results[n]["out"].reshape(C, H, W).astype(np.float32)
                    for n in range(N)])
    if _trace:
        kernel._last_results = res
    return out
